# revision 1
# baseline (speedup 1.0000x reference)
"""2-layer GCN (gnn_message_passing) on 8 Trainium2 NeuronCores — v3.

Source-sharded: each core owns 12500 nodes (features + support rows local).
Per layer: support = X_c @ W (local, PE) -> local HBM table -> per GLOBAL dest
block: dma_gather local source rows (edges bucketed by dest block on host,
int16 local indices), scatter into the block via one-hot matmul in PSUM ->
partial-output tables (per dest quarter) -> chunked ReduceScatter(add) sums
the 8 cores' partials; each core receives its own 12500 rows. ReLU + W2
transform after RS1; layer-2 scatter reuses the same edge buffers (same edge
order) with width 64. Collectives are out-small (RS) and overlap the scatter
pipeline via per-quarter tensors.
"""
import sys
sys.path.insert(0, "/opt/trn_rl_repo")

import numpy as np
from contextlib import ExitStack

import concourse.bass as bass
import concourse.bacc as bacc
import concourse.tile as tile
from concourse import bass_utils
from concourse import mybir
from concourse.library_config import mlp

PADVAL = 200.0
GMAX = 8   # max 128-idx chunks per dma_gather call (HW limit: 1024 idx)
F = 4      # dest blocks per supergroup
NQ = 7     # ReduceScatter chunks (dest sevenths; 14 blocks each, even for pairing)


class Config:
    def __init__(self, n=100000, in_dim=256, hid=128, out_dim=64, ncore=8):
        self.N = n
        self.IN = in_dim
        self.HID = hid
        self.OUT = out_dim
        self.NCORE = ncore
        self.NPC = n // ncore
        assert self.NPC * ncore == n
        self.NB = (self.NPC + 127) // 128          # 98 local blocks
        self.NPP = self.NB * 128                   # 12544
        self.GB = ncore * self.NB                  # 784 global dest blocks
        self.KT = in_dim // 128
        # quarter sizes in local blocks: quad-packable regions + runt
        self.QB = [16] * 6 + [2]
        assert sum(self.QB) == self.NB and len(self.QB) == NQ
        self.QSTART = [sum(self.QB[:q]) for q in range(NQ)]
        assert self.NPP <= 32767  # int16 gather idx


CFG = Config()


def prep_edges(cfg, edge_index):
    """Bucket each core's SOURCE-owned edges by global dest block in the
    quarter-major P-table order. Returns per-core colidx (16-wrapped int16
    local source row), rowloc (fp16 dest-row-in-block), uniform chunk counts
    per P-block, and chunk offsets."""
    c = cfg
    row = np.asarray(edge_index[0], dtype=np.int64)   # dest
    col = np.asarray(edge_index[1], dtype=np.int64)   # src
    score = col // c.NPC                              # owner core (source)
    sloc = (col - score * c.NPC).astype(np.int16)     # gather row in t-table
    dcore = row // c.NPC
    dl = row - dcore * c.NPC
    db = dl // 128                                    # dest local block
    rib = (dl % 128).astype(np.float16)
    # quarter of dest block
    qid = np.searchsorted(np.asarray(c.QSTART + [c.NB]), db, side="right") - 1
    # P-block index: quarter-major, core-major inside
    qb = np.asarray(c.QB)[qid]
    blocks_before = np.asarray([sum(c.QB[:q]) for q in range(NQ)])[qid] * c.NCORE
    pblk = blocks_before + dcore * qb + (db - np.asarray(c.QSTART)[qid])

    key = score * c.GB + pblk
    order = np.argsort(key, kind="stable")
    key_s = key[order]
    sloc_s = sloc[order]
    rib_s = rib[order]
    counts = np.bincount(key_s, minlength=c.NCORE * c.GB).reshape(
        c.NCORE, c.GB)
    starts = np.concatenate([[0], np.cumsum(counts.reshape(-1))])

    chunks = -(-counts.max(axis=0) // 128)            # (GB,) uniform chunks
    nch = int(chunks.sum())
    cw = nch * 8
    qoff = np.concatenate([[0], np.cumsum(chunks)])   # chunk offset per block

    colidx = np.zeros((c.NCORE, 128, cw), np.int16)
    rowloc = np.full((c.NCORE, 128, nch), PADVAL, np.float16)
    for cc in range(c.NCORE):
        for p in range(c.GB):
            nk = int(chunks[p])
            if nk == 0:
                continue
            ki = cc * c.GB + p
            s, e = starts[ki], starts[ki + 1]
            cnt = e - s
            cap = nk * 128
            tl = np.zeros(cap, np.int16)
            tl[:cnt] = sloc_s[s:e]
            rb = np.full(cap, PADVAL, np.float16)
            rb[:cnt] = rib_s[s:e]
            qo = int(qoff[p])
            colidx[cc][:, qo * 8:(qo + nk) * 8] = np.tile(
                tl.reshape(-1, 16).T, (8, 1))
            rowloc[cc][:, qo:qo + nk] = rb.reshape(nk, 128).T
    cnt128 = counts.max(axis=0)                       # (GB,) true max counts
    return colidx, rowloc, chunks, qoff, nch, cw, cnt128


def build(cfg, chunks, qoff, nch, cw, cnt128):
    c = cfg
    nc = bacc.Bacc(None, target_bir_lowering=False, debug=False,
                   num_devices=c.NCORE, name="gcnv3", num_swdge_queues=4)
    f16, f32, i16 = mybir.dt.float16, mybir.dt.float32, mybir.dt.int16
    relu = mybir.ActivationFunctionType.Relu
    copyf = mybir.ActivationFunctionType.Copy

    xT = nc.dram_tensor("xT", (c.IN, c.NPP), f16, kind="ExternalInput")
    w1 = nc.dram_tensor("w1", (c.IN, c.HID), f16, kind="ExternalInput")
    w2p = nc.dram_tensor("w2p", (c.HID, 128), f16, kind="ExternalInput")
    iota = nc.dram_tensor("iota", (128, 128), f16, kind="ExternalInput")
    ident = nc.dram_tensor("ident", (128, 128), f16, kind="ExternalInput")
    colidx = nc.dram_tensor("colidx", (128, cw), i16, kind="ExternalInput")
    rowloc = nc.dram_tensor("rowloc", (128, nch), f16, kind="ExternalInput")
    out = nc.dram_tensor("out", (c.NPC, c.OUT), f32, kind="ExternalOutput")

    t1 = nc.dram_tensor("t1", (c.NPP, c.HID), f16, kind="Internal")
    t2 = nc.dram_tensor("t2", (c.NPP, 128), f16, kind="Internal")
    # per-quarter partial tables + RS outputs (separate tensors so each RS
    # only waits on its own quarter's writers)
    # L1 partials pack two dest blocks per row (512B rows -> full-rate DMA)
    P1 = [nc.dram_tensor(f"P1q{q}", (c.NCORE * (c.QB[q] // 2) * 128, 256), f16,
                         kind="Internal") for q in range(NQ)]
    H1 = [nc.dram_tensor(f"H1q{q}", ((c.QB[q] // 2) * 128, 256), f16,
                         kind="Internal") for q in range(NQ)]
    # L2 partials pack 4 (or 2, runt quarter) dest blocks per row
    pk2 = [4 if c.QB[q] % 4 == 0 else 2 for q in range(NQ)]
    P2 = [nc.dram_tensor(f"P2q{q}",
                         (c.NCORE * (c.QB[q] // pk2[q]) * 128,
                          c.OUT * pk2[q]), f16,
                         kind="Internal") for q in range(NQ)]
    O2 = [nc.dram_tensor(f"O2q{q}",
                         ((c.QB[q] // pk2[q]) * 128, c.OUT * pk2[q]), f16,
                         kind="Internal") for q in range(NQ)]

    groups = [list(range(c.NCORE))]
    iseq = mybir.AluOpType.is_equal

    # P-block -> (quarter, core, block-in-quarter) and row offset in P[q]
    def pmeta(p):
        acc = 0
        for q in range(NQ):
            nb = c.NCORE * c.QB[q]
            if p < acc + nb:
                j = p - acc
                return q, j * 128  # row offset within P[q]
            acc += nb
        raise AssertionError

    with ExitStack() as ctx:
        tc = ctx.enter_context(tile.TileContext(nc))
        nc.gpsimd.load_library(mlp)
        cpool = ctx.enter_context(tc.tile_pool(name="const", bufs=1))
        # shared gather pool, warmed once (hidden under phase A): trimmed
        # calls leave tail rows unwritten, which must be stale-finite
        pg = ctx.enter_context(tc.tile_pool(name="pg", bufs=24))
        for _ in range(24):
            wt = pg.tile((128, GMAX, 128), mybir.dt.float16, name="gt")
            nc.vector.memset(wt[:], 0.0)
        iota_sb = cpool.tile((128, 128), f16, tag="iota")
        nc.sync.dma_start(iota_sb[:], iota[:])
        ident_sb = cpool.tile((128, 128), f16, tag="ident")
        nc.sync.dma_start(ident_sb[:], ident[:])
        w2_sb = cpool.tile((c.HID, 128), f16, tag="w2")
        nc.sync.dma_start(w2_sb[:], w2p[:])

        # Phase A: t1 = X_c @ W1 (local support)
        with tc.tile_pool(name="pa", bufs=1) as pa, \
             tc.tile_pool(name="pas", bufs=3) as pas, \
             tc.tile_pool(name="psa", bufs=2, space="PSUM") as psa:
            xk, w1k = [], []
            half = (c.NB // 2) * 128
            for k in range(c.KT):
                t = pa.tile((128, c.HID), f16, tag=f"w{k}", name=f"w1k{k}")
                nc.sync.dma_start(t[:], w1.ap()[k * 128:(k + 1) * 128, :])
                w1k.append(t)
            for k in range(c.KT):
                # column halves: block matmuls start at 50% of the x load
                ta = pa.tile((128, half), f16, tag=f"xa{k}", name=f"xka{k}")
                nc.sync.dma_start(ta[:],
                                  xT.ap()[k * 128:(k + 1) * 128, 0:half])
                tb = pa.tile((128, c.NPP - half), f16, tag=f"xb{k}",
                             name=f"xkb{k}")
                nc.sync.dma_start(tb[:],
                                  xT.ap()[k * 128:(k + 1) * 128, half:])
                xk.append((ta, tb))
            for b0 in range(0, c.NB, 2):
                gn = min(2, c.NB - b0)
                s1 = pas.tile((128, 2, c.HID), f16, name='s1')
                for i in range(gn):
                    b = b0 + i
                    ps = psa.tile((128, c.HID), f32, space="PSUM")
                    for k in range(c.KT):
                        xt_, off = ((xk[k][0], 0) if (b + 1) * 128 <= half
                                    else (xk[k][1], half))
                        nc.tensor.matmul(
                            ps[:],
                            xt_[:, b * 128 - off:(b + 1) * 128 - off],
                            w1k[k][:], start=(k == 0), stop=(k == c.KT - 1))
                    nc.scalar.activation(s1[:, i, :], ps[:], copyf)
                nc.sync.dma_start(
                    t1.ap()[b0 * 128:(b0 + gn) * 128, :]
                    .rearrange("(blk p) f -> p blk f", blk=gn),
                    s1[:, 0:gn, :])

        # edge metadata loads queued after phase-A inputs (same SP DMA FIFO)
        rowloc_sb = cpool.tile((128, nch), f16, tag="rowloc")
        nc.sync.dma_start(rowloc_sb[:], rowloc[:])
        colidx_sb = cpool.tile((128, cw), i16, tag="colidx")
        nc.sync.dma_start(colidx_sb[:], colidx[:])

        # chunk index -> trailing-pad trim when a gather call ends exactly at
        # this chunk boundary (a cell's last chunk): skip the cell's pad
        # slots.  num_idxs need not be a multiple of 128; untouched tail rows
        # of the (reused, uniform-shape) pool buffers hold stale-but-finite
        # data that the zero one-hot columns nullify.  The first 24 calls per
        # layer stay untrimmed so every pool buffer is fully written once.
        endtrim = {}
        for b in range(c.GB):
            if chunks[b] > 0:
                endtrim[int(qoff[b + 1])] = int(chunks[b] * 128 - cnt128[b])

        # P-block index at which each quarter region ends
        qends = {}
        acc = 0
        for q in range(NQ):
            acc += c.NCORE * c.QB[q]
            qends[acc] = q

        def scatter_layer(tab, width, emit, qhook=None):
            """Gather+scatter all P-blocks from local table `tab`;
            emit(p, psum_ap) per finished block; qhook(q) fires inline after
            the last supergroup of quarter q so collectives dispatch from the
            Pool sequencer mid-layer instead of queuing behind all gathers."""
            gq = [0]
            with tc.tile_pool(name="poh", bufs=3) as poh, \
                 tc.tile_pool(name="pso", bufs=2, space="PSUM") as pso:
                for g0 in range(0, c.GB, F):
                    bs = list(range(g0, min(g0 + F, c.GB)))
                    wg = int(chunks[bs].sum())
                    qg0 = int(qoff[bs[0]])
                    cblk = [b for b in bs for _ in range(int(chunks[b]))]
                    first, last = {}, {}
                    for ci, b in enumerate(cblk):
                        first.setdefault(b, ci)
                        last[b] = ci
                    psb = {b: pso.tile((128, width), f32, space="PSUM",
                                       tag=f"ps{b - g0}", name=f"ps{b - g0}")
                           for b in bs}
                    for b in bs:
                        if b not in first:
                            zt = pg.tile((128, width), f16)
                            nc.vector.memset(zt[:], 0.0)
                            nc.tensor.matmul(psb[b][:], ident_sb[:, 0:width],
                                             zt[:], start=True, stop=True)
                    if wg > 0:
                        oh = poh.tile((128, wg, 128), f16)
                        nc.vector.tensor_tensor(
                            out=oh[:],
                            in0=rowloc_sb[:, qg0:qg0 + wg].unsqueeze(2)
                                .to_broadcast((128, wg, 128)),
                            in1=iota_sb[:].unsqueeze(1)
                                .to_broadcast((128, wg, 128)),
                            op=iseq)
                        # choose call windows (<= GMAX chunks, minimal
                        # count) maximizing endings on cell boundaries so
                        # their trailing pads can be trimmed
                        K = -(-wg // GMAX)
                        dpk = {0: (0, [])}
                        ends = None
                        for _ in range(K):
                            nxt = {}
                            for pos, (t, es) in dpk.items():
                                for e in range(pos + 1,
                                               min(pos + GMAX, wg) + 1):
                                    tt = t + (1 if (qg0 + e) in endtrim
                                              else 0)
                                    if e not in nxt or nxt[e][0] < tt:
                                        nxt[e] = (tt, es + [e])
                            dpk = nxt
                            if wg in dpk:
                                ends = dpk[wg][1]
                                break
                        assert ends is not None
                        for wi, e in enumerate(ends):
                            s0 = ends[wi - 1] if wi else 0
                            sn = e - s0
                            trim = endtrim.get(qg0 + e, 0)
                            nidx = sn * 128 - trim
                            gt = pg.tile((128, GMAX, 128), f16, name="gt")
                            nc.gpsimd.dma_gather(
                                gt[:, 0:sn, :], tab.ap(),
                                colidx_sb[:, (qg0 + s0) * 8:
                                          (qg0 + s0 + sn) * 8],
                                nidx, nidx, 128,
                                queue_num=gq[0] % 4)
                            gq[0] += 1
                            for j in range(sn):
                                ci = s0 + j
                                b = cblk[ci]
                                nc.tensor.matmul(
                                    psb[b][:], oh[:, ci, :],
                                    gt[:, j, 0:width],
                                    start=(ci == first[b]),
                                    stop=(ci == last[b]))
                    for b in bs:
                        emit(b, psb[b])
                    if qhook is not None and (g0 + F) in qends:
                        qhook(qends[g0 + F])

        # Layer-1 scatter -> P1 partials (pair-packed, batched per supergroup)
        with tc.tile_pool(name="pe1", bufs=4) as pe1:
            st1 = [None]

            def emit1(p, ps):
                i = p % F
                if i == 0:
                    st1[0] = pe1.tile((128, F // 2, 256), f16, name='st1')
                nc.scalar.activation(
                    st1[0][:, i // 2, (i % 2) * 128:(i % 2 + 1) * 128],
                    ps[:], copyf)
                if i == F - 1:
                    q, roff = pmeta(p - F + 1)
                    prow = roff // 2
                    nc.sync.dma_start(
                        P1[q].ap()[prow:prow + (F // 2) * 128, :]
                        .rearrange("(hi p) f -> p hi f", hi=F // 2), st1[0][:])
            def rs1(q):
                nc.gpsimd.collective_compute(
                    "ReduceScatter", mybir.AluOpType.add,
                    replica_groups=groups,
                    ins=[P1[q].ap()], outs=[H1[q].ap()])
            scatter_layer(t1, c.HID, emit1, qhook=rs1)

        # Phase B2: per local block: relu(h1) -> transpose -> @W2 -> t2
        # (loads/stores batched 4 blocks per DMA within each quarter)
        with tc.tile_pool(name="ph1", bufs=3) as ph1, \
             tc.tile_pool(name="pds", bufs=3) as pds, \
             tc.tile_pool(name="pst", bufs=2, space="PSUM") as pst, \
             tc.tile_pool(name="psd", bufs=2, space="PSUM") as psd:
            for q in range(NQ):
                for lb0 in range(0, c.QB[q], 4):
                    gn = min(4, c.QB[q] - lb0)
                    nhi = gn // 2
                    hb = ph1.tile((128, 2, 256), f16, tag="hb", name="hb")
                    nc.sync.dma_start(
                        hb[:, 0:nhi, :],
                        H1[q].ap()[(lb0 // 2) * 128:(lb0 // 2 + nhi) * 128, :]
                        .rearrange("(hi p) f -> p hi f", hi=nhi))
                    s2 = pds.tile((128, 4, 128), f16, name='s2')
                    h1b4 = ph1.tile((128, 2, 256), f16, tag="h1b",
                                    name="h1b")
                    nc.scalar.activation(h1b4[:, 0:nhi, :],
                                         hb[:, 0:nhi, :], relu)
                    for i in range(gn):
                        tp = pst.tile((c.HID, 128), f16, space="PSUM")
                        nc.tensor.transpose(
                            out=tp[:],
                            in_=h1b4[:, i // 2,
                                     (i % 2) * 128:(i % 2 + 1) * 128],
                            identity=ident_sb[:])
                        h1t = ph1.tile((c.HID, 128), f16, tag="h1t",
                                       name="h1t")
                        nc.vector.tensor_copy(h1t[:], tp[:])
                        ps2 = psd.tile((128, 128), f32, space="PSUM")
                        nc.tensor.matmul(ps2[:], h1t[:], w2_sb[:],
                                         start=True, stop=True)
                        if i % 2 == 0:
                            nc.scalar.activation(s2[:, i, :], ps2[:], copyf)
                        else:
                            nc.vector.tensor_copy(s2[:, i, :], ps2[:])
                    b0 = c.QSTART[q] + lb0
                    nc.sync.dma_start(
                        t2.ap()[b0 * 128:(b0 + gn) * 128, :]
                        .rearrange("(blk p) f -> p blk f", blk=gn),
                        s2[:, 0:gn, :])

        # Layer-2 scatter -> P2 partials (quad/pair-packed rows)
        with tc.tile_pool(name="pe2", bufs=4) as pe2:
            st2 = [None]

            def emit2(p, ps):
                q, roff = pmeta(p)
                pk = pk2[q]
                i = p % pk
                if i == 0:
                    st2[0] = pe2.tile((128, 256), f16, name='st2')
                nc.scalar.activation(
                    st2[0][:, i * c.OUT:(i + 1) * c.OUT], ps[:], copyf)
                if i == pk - 1:
                    grow = (roff // pk) - (pk - 1) * 128 // pk
                    nc.sync.dma_start(
                        P2[q].ap()[grow:grow + 128, :],
                        st2[0][:, 0:pk * c.OUT])
            def rs2(q):
                nc.gpsimd.collective_compute(
                    "ReduceScatter", mybir.AluOpType.add,
                    replica_groups=groups,
                    ins=[P2[q].ap()], outs=[O2[q].ap()])
            scatter_layer(t2, c.OUT, emit2, qhook=rs2)

        # Final: O2 -> fp32 out (packed loads; last block is short)
        with tc.tile_pool(name="po", bufs=3) as po:
            for q in range(NQ):
                pk = pk2[q]
                for lb0 in range(0, c.QB[q], pk):
                    gn = pk
                    ot = po.tile((128, 4, c.OUT), f16, tag="ot", name="ot")
                    nc.sync.dma_start(
                        ot[:, 0:gn, :].rearrange("p blk f -> p (blk f)"),
                        O2[q].ap()[(lb0 // pk) * 128:
                                   (lb0 // pk + 1) * 128, :])
                    of = po.tile((128, 4, c.OUT), f32, tag="of", name="of")
                    nc.scalar.activation(of[:, 0:gn, :], ot[:, 0:gn, :], copyf)
                    b0 = c.QSTART[q] + lb0
                    full = min(gn, max(0, (c.NPC // 128) - b0))
                    if full > 0:
                        nc.sync.dma_start(
                            out.ap()[b0 * 128:(b0 + full) * 128, :]
                            .rearrange("(blk p) f -> p blk f", blk=full),
                            of[:, 0:full, :])
                    for i in range(full, gn):
                        b = b0 + i
                        rows = min(128, c.NPC - b * 128)
                        if rows > 0:
                            nc.sync.dma_start(
                                out.ap()[b * 128:b * 128 + rows, :],
                                of[0:rows, i, :])

    nc.compile()
    return nc


def make_inputs(cfg, features, edge_index, W1, W2):
    c = cfg
    colidx, rowloc, chunks, qoff, nch, cw, cnt128 = prep_edges(
        cfg, edge_index)
    iota2d = np.broadcast_to(np.arange(128, dtype=np.float16),
                             (128, 128)).copy()
    ident = np.eye(128, dtype=np.float16)
    w1 = np.ascontiguousarray(np.asarray(W1, np.float16))
    w2pad = np.zeros((c.HID, 128), np.float16)
    w2pad[:, :c.OUT] = np.asarray(W2, np.float16)
    in_maps = []
    for cc in range(c.NCORE):
        xc = np.asarray(features[cc * c.NPC:(cc + 1) * c.NPC], np.float32)
        xt = np.zeros((c.IN, c.NPP), np.float16)
        xt[:, :c.NPC] = xc.T.astype(np.float16)
        in_maps.append({
            "xT": np.ascontiguousarray(xt),
            "w1": w1, "w2p": w2pad, "iota": iota2d, "ident": ident,
            "colidx": np.ascontiguousarray(colidx[cc]),
            "rowloc": np.ascontiguousarray(rowloc[cc]),
        })
    return in_maps, chunks, qoff, nch, cw, cnt128


_LAST_NC = None


def kernel(features, edge_index, W1, W2):
    global _LAST_NC
    cfg = CFG
    in_maps, chunks, qoff, nch, cw, cnt128 = make_inputs(
        cfg, features, edge_index, W1, W2)
    nc = build(cfg, chunks, qoff, nch, cw, cnt128)
    _LAST_NC = nc
    res = bass_utils.run_bass_kernel_spmd(
        nc, in_maps, core_ids=list(range(cfg.NCORE)))
    return np.concatenate(
        [res.results[cc]["out"] for cc in range(cfg.NCORE)], axis=0)



# revision 4
# speedup vs baseline: 1.0093x; 1.0093x over previous
"""2-layer GCN (gnn_message_passing) on 8 Trainium2 NeuronCores — v3.

Source-sharded: each core owns 12500 nodes (features + support rows local).
Per layer: support = X_c @ W (local, PE) -> local HBM table -> per GLOBAL dest
block: dma_gather local source rows (edges bucketed by dest block on host,
int16 local indices), scatter into the block via one-hot matmul in PSUM ->
partial-output tables (per dest quarter) -> chunked ReduceScatter(add) sums
the 8 cores' partials; each core receives its own 12500 rows. ReLU + W2
transform after RS1; layer-2 scatter reuses the same edge buffers (same edge
order) with width 64. Collectives are out-small (RS) and overlap the scatter
pipeline via per-quarter tensors.
"""
import sys
sys.path.insert(0, "/opt/trn_rl_repo")

import numpy as np
from contextlib import ExitStack

import concourse.bass as bass
import concourse.bacc as bacc
import concourse.tile as tile
from concourse import bass_utils
from concourse import mybir
from concourse.library_config import mlp

PADVAL = 200.0
GMAX = 8   # max 128-idx chunks per dma_gather call (HW limit: 1024 idx)
F = 4      # dest blocks per supergroup
NQ = 7     # ReduceScatter chunks (dest sevenths; 14 blocks each, even for pairing)


class Config:
    def __init__(self, n=100000, in_dim=256, hid=128, out_dim=64, ncore=8):
        self.N = n
        self.IN = in_dim
        self.HID = hid
        self.OUT = out_dim
        self.NCORE = ncore
        self.NPC = n // ncore
        assert self.NPC * ncore == n
        self.NB = (self.NPC + 127) // 128          # 98 local blocks
        self.NPP = self.NB * 128                   # 12544
        self.GB = ncore * self.NB                  # 784 global dest blocks
        self.KT = in_dim // 128
        # quarter sizes in local blocks: quad-packable regions + runt
        self.QB = [16] * 6 + [2]
        assert sum(self.QB) == self.NB and len(self.QB) == NQ
        self.QSTART = [sum(self.QB[:q]) for q in range(NQ)]
        assert self.NPP <= 32767  # int16 gather idx


CFG = Config()


def prep_edges(cfg, edge_index):
    """Bucket each core's SOURCE-owned edges by global dest block in the
    quarter-major P-table order. Returns per-core colidx (16-wrapped int16
    local source row), rowloc (fp16 dest-row-in-block), uniform chunk counts
    per P-block, and chunk offsets."""
    c = cfg
    row = np.asarray(edge_index[0], dtype=np.int64)   # dest
    col = np.asarray(edge_index[1], dtype=np.int64)   # src
    score = col // c.NPC                              # owner core (source)
    sloc = (col - score * c.NPC).astype(np.int16)     # gather row in t-table
    dcore = row // c.NPC
    dl = row - dcore * c.NPC
    db = dl // 128                                    # dest local block
    rib = (dl % 128).astype(np.float16)
    # quarter of dest block
    qid = np.searchsorted(np.asarray(c.QSTART + [c.NB]), db, side="right") - 1
    # P-block index: quarter-major, core-major inside
    qb = np.asarray(c.QB)[qid]
    blocks_before = np.asarray([sum(c.QB[:q]) for q in range(NQ)])[qid] * c.NCORE
    pblk = blocks_before + dcore * qb + (db - np.asarray(c.QSTART)[qid])

    key = score * c.GB + pblk
    order = np.argsort(key, kind="stable")
    key_s = key[order]
    sloc_s = sloc[order]
    rib_s = rib[order]
    counts = np.bincount(key_s, minlength=c.NCORE * c.GB).reshape(
        c.NCORE, c.GB)
    starts = np.concatenate([[0], np.cumsum(counts.reshape(-1))])

    chunks = -(-counts.max(axis=0) // 128)            # (GB,) uniform chunks
    nch = int(chunks.sum())
    cw = nch * 8
    qoff = np.concatenate([[0], np.cumsum(chunks)])   # chunk offset per block

    colidx = np.zeros((c.NCORE, 128, cw), np.int16)
    rowloc = np.full((c.NCORE, 128, nch), PADVAL, np.float16)
    for cc in range(c.NCORE):
        for p in range(c.GB):
            nk = int(chunks[p])
            if nk == 0:
                continue
            ki = cc * c.GB + p
            s, e = starts[ki], starts[ki + 1]
            cnt = e - s
            cap = nk * 128
            tl = np.zeros(cap, np.int16)
            tl[:cnt] = sloc_s[s:e]
            rb = np.full(cap, PADVAL, np.float16)
            rb[:cnt] = rib_s[s:e]
            qo = int(qoff[p])
            colidx[cc][:, qo * 8:(qo + nk) * 8] = np.tile(
                tl.reshape(-1, 16).T, (8, 1))
            rowloc[cc][:, qo:qo + nk] = rb.reshape(nk, 128).T
    cnt128 = counts.max(axis=0)                       # (GB,) true max counts
    return colidx, rowloc, chunks, qoff, nch, cw, cnt128


def build(cfg, chunks, qoff, nch, cw, cnt128):
    c = cfg
    nc = bacc.Bacc(None, target_bir_lowering=False, debug=False,
                   num_devices=c.NCORE, name="gcnv3", num_swdge_queues=4)
    f16, f32, i16 = mybir.dt.float16, mybir.dt.float32, mybir.dt.int16
    relu = mybir.ActivationFunctionType.Relu
    copyf = mybir.ActivationFunctionType.Copy

    xT = nc.dram_tensor("xT", (c.IN, c.NPP), f16, kind="ExternalInput")
    w1 = nc.dram_tensor("w1", (c.IN, c.HID), f16, kind="ExternalInput")
    w2p = nc.dram_tensor("w2p", (c.HID, 128), f16, kind="ExternalInput")
    iota = nc.dram_tensor("iota", (128, 128), f16, kind="ExternalInput")
    ident = nc.dram_tensor("ident", (128, 128), f16, kind="ExternalInput")
    colidx = nc.dram_tensor("colidx", (128, cw), i16, kind="ExternalInput")
    rowloc = nc.dram_tensor("rowloc", (128, nch), f16, kind="ExternalInput")
    out = nc.dram_tensor("out", (c.NPC, c.OUT), f32, kind="ExternalOutput")

    t1 = nc.dram_tensor("t1", (c.NPP, c.HID), f16, kind="Internal")
    t2 = nc.dram_tensor("t2", (c.NPP, 128), f16, kind="Internal")
    # per-quarter partial tables + RS outputs (separate tensors so each RS
    # only waits on its own quarter's writers)
    # L1 partials pack two dest blocks per row (512B rows -> full-rate DMA)
    P1 = [nc.dram_tensor(f"P1q{q}", (c.NCORE * (c.QB[q] // 2) * 128, 256), f16,
                         kind="Internal") for q in range(NQ)]
    H1 = [nc.dram_tensor(f"H1q{q}", ((c.QB[q] // 2) * 128, 256), f16,
                         kind="Internal") for q in range(NQ)]
    # L2 partials pack 4 (or 2, runt quarter) dest blocks per row
    pk2 = [4 if c.QB[q] % 4 == 0 else 2 for q in range(NQ)]
    P2 = [nc.dram_tensor(f"P2q{q}",
                         (c.NCORE * (c.QB[q] // pk2[q]) * 128,
                          c.OUT * pk2[q]), f16,
                         kind="Internal") for q in range(NQ)]
    O2 = [nc.dram_tensor(f"O2q{q}",
                         ((c.QB[q] // pk2[q]) * 128, c.OUT * pk2[q]), f16,
                         kind="Internal") for q in range(NQ)]

    groups = [list(range(c.NCORE))]
    iseq = mybir.AluOpType.is_equal

    # P-block -> (quarter, core, block-in-quarter) and row offset in P[q]
    def pmeta(p):
        acc = 0
        for q in range(NQ):
            nb = c.NCORE * c.QB[q]
            if p < acc + nb:
                j = p - acc
                return q, j * 128  # row offset within P[q]
            acc += nb
        raise AssertionError

    with ExitStack() as ctx:
        tc = ctx.enter_context(tile.TileContext(nc))
        nc.gpsimd.load_library(mlp)
        cpool = ctx.enter_context(tc.tile_pool(name="const", bufs=1))
        # shared gather pool, warmed once (hidden under phase A): trimmed
        # calls leave tail rows unwritten, which must be stale-finite
        pg = ctx.enter_context(tc.tile_pool(name="pg", bufs=24))
        for _ in range(24):
            wt = pg.tile((128, GMAX, 128), mybir.dt.float16, name="gt")
            nc.vector.memset(wt[:], 0.0)
        iota_sb = cpool.tile((128, 128), f16, tag="iota")
        nc.sync.dma_start(iota_sb[:], iota[:])
        ident_sb = cpool.tile((128, 128), f16, tag="ident")
        nc.sync.dma_start(ident_sb[:], ident[:])
        w2_sb = cpool.tile((c.HID, 128), f16, tag="w2")
        nc.sync.dma_start(w2_sb[:], w2p[:])

        # Phase A: t1 = X_c @ W1 (local support)
        with tc.tile_pool(name="pa", bufs=1) as pa, \
             tc.tile_pool(name="pas", bufs=3) as pas, \
             tc.tile_pool(name="psa", bufs=2, space="PSUM") as psa:
            xk, w1k = [], []
            half = (c.NB // 2) * 128
            for k in range(c.KT):
                t = pa.tile((128, c.HID), f16, tag=f"w{k}", name=f"w1k{k}")
                nc.sync.dma_start(t[:], w1.ap()[k * 128:(k + 1) * 128, :])
                w1k.append(t)
            for k in range(c.KT):
                # column halves: block matmuls start at 50% of the x load
                ta = pa.tile((128, half), f16, tag=f"xa{k}", name=f"xka{k}")
                nc.sync.dma_start(ta[:],
                                  xT.ap()[k * 128:(k + 1) * 128, 0:half])
                tb = pa.tile((128, c.NPP - half), f16, tag=f"xb{k}",
                             name=f"xkb{k}")
                nc.sync.dma_start(tb[:],
                                  xT.ap()[k * 128:(k + 1) * 128, half:])
                xk.append((ta, tb))
            for b0 in range(0, c.NB, 2):
                gn = min(2, c.NB - b0)
                s1 = pas.tile((128, 2, c.HID), f16, name='s1')
                for i in range(gn):
                    b = b0 + i
                    ps = psa.tile((128, c.HID), f32, space="PSUM")
                    for k in range(c.KT):
                        xt_, off = ((xk[k][0], 0) if (b + 1) * 128 <= half
                                    else (xk[k][1], half))
                        nc.tensor.matmul(
                            ps[:],
                            xt_[:, b * 128 - off:(b + 1) * 128 - off],
                            w1k[k][:], start=(k == 0), stop=(k == c.KT - 1))
                    nc.scalar.activation(s1[:, i, :], ps[:], copyf)
                nc.sync.dma_start(
                    t1.ap()[b0 * 128:(b0 + gn) * 128, :]
                    .rearrange("(blk p) f -> p blk f", blk=gn),
                    s1[:, 0:gn, :])

        # edge metadata loads queued after phase-A inputs (same SP DMA FIFO)
        rowloc_sb = cpool.tile((128, nch), f16, tag="rowloc")
        nc.sync.dma_start(rowloc_sb[:], rowloc[:])
        colidx_sb = cpool.tile((128, cw), i16, tag="colidx")
        nc.sync.dma_start(colidx_sb[:], colidx[:])

        # chunk index -> trailing-pad trim when a gather call ends exactly at
        # this chunk boundary (a cell's last chunk): skip the cell's pad
        # slots.  num_idxs need not be a multiple of 128; untouched tail rows
        # of the (reused, uniform-shape) pool buffers hold stale-but-finite
        # data that the zero one-hot columns nullify.  The first 24 calls per
        # layer stay untrimmed so every pool buffer is fully written once.
        endtrim = {}
        for b in range(c.GB):
            if chunks[b] > 0:
                endtrim[int(qoff[b + 1])] = int(chunks[b] * 128 - cnt128[b])

        # P-block index at which each quarter region ends
        qends = {}
        acc = 0
        for q in range(NQ):
            acc += c.NCORE * c.QB[q]
            qends[acc] = q

        def scatter_layer(tab, width, emit, qhook=None):
            """Gather+scatter all P-blocks from local table `tab`;
            emit(p, psum_ap) per finished block; qhook(q) fires inline after
            the last supergroup of quarter q so collectives dispatch from the
            Pool sequencer mid-layer instead of queuing behind all gathers."""
            gq = [0]
            with tc.tile_pool(name="poh", bufs=3) as poh, \
                 tc.tile_pool(name="pso", bufs=2, space="PSUM") as pso:
                for g0 in range(0, c.GB, F):
                    bs = list(range(g0, min(g0 + F, c.GB)))
                    wg = int(chunks[bs].sum())
                    qg0 = int(qoff[bs[0]])
                    cblk = [b for b in bs for _ in range(int(chunks[b]))]
                    first, last = {}, {}
                    for ci, b in enumerate(cblk):
                        first.setdefault(b, ci)
                        last[b] = ci
                    psb = {b: pso.tile((128, width), f32, space="PSUM",
                                       tag=f"ps{b - g0}", name=f"ps{b - g0}")
                           for b in bs}
                    for b in bs:
                        if b not in first:
                            zt = pg.tile((128, width), f16)
                            nc.vector.memset(zt[:], 0.0)
                            nc.tensor.matmul(psb[b][:], ident_sb[:, 0:width],
                                             zt[:], start=True, stop=True)
                    if wg > 0:
                        oh = poh.tile((128, wg, 128), f16)
                        nc.vector.tensor_tensor(
                            out=oh[:],
                            in0=rowloc_sb[:, qg0:qg0 + wg].unsqueeze(2)
                                .to_broadcast((128, wg, 128)),
                            in1=iota_sb[:].unsqueeze(1)
                                .to_broadcast((128, wg, 128)),
                            op=iseq)
                        # choose call windows (<= GMAX chunks, minimal
                        # count) maximizing endings on cell boundaries so
                        # their trailing pads can be trimmed
                        K = -(-wg // GMAX)
                        dpk = {0: (0, [])}
                        ends = None
                        for _ in range(K):
                            nxt = {}
                            for pos, (t, es) in dpk.items():
                                for e in range(pos + 1,
                                               min(pos + GMAX, wg) + 1):
                                    tt = t + (1 if (qg0 + e) in endtrim
                                              else 0)
                                    if e not in nxt or nxt[e][0] < tt:
                                        nxt[e] = (tt, es + [e])
                            dpk = nxt
                            if wg in dpk:
                                ends = dpk[wg][1]
                                break
                        assert ends is not None
                        for wi, e in enumerate(ends):
                            s0 = ends[wi - 1] if wi else 0
                            sn = e - s0
                            trim = endtrim.get(qg0 + e, 0)
                            nidx = sn * 128 - trim
                            gt = pg.tile((128, GMAX, 128), f16, name="gt")
                            nc.gpsimd.dma_gather(
                                gt[:, 0:sn, :], tab.ap(),
                                colidx_sb[:, (qg0 + s0) * 8:
                                          (qg0 + s0 + sn) * 8],
                                nidx, nidx, 128,
                                queue_num=gq[0] % 4)
                            gq[0] += 1
                            for j in range(sn):
                                ci = s0 + j
                                b = cblk[ci]
                                nc.tensor.matmul(
                                    psb[b][:], oh[:, ci, :],
                                    gt[:, j, 0:width],
                                    start=(ci == first[b]),
                                    stop=(ci == last[b]))
                    for b in bs:
                        emit(b, psb[b])
                    if qhook is not None and (g0 + F) in qends:
                        qhook(qends[g0 + F])

        # Layer-1 scatter -> P1 partials (pair-packed, batched per supergroup)
        with tc.tile_pool(name="pe1", bufs=4) as pe1:
            st1 = [None]

            def emit1(p, ps):
                i = p % F
                if i == 0:
                    st1[0] = pe1.tile((128, F // 2, 256), f16, name='st1')
                nc.scalar.activation(
                    st1[0][:, i // 2, (i % 2) * 128:(i % 2 + 1) * 128],
                    ps[:], copyf)
                if i == F - 1:
                    q, roff = pmeta(p - F + 1)
                    prow = roff // 2
                    nc.sync.dma_start(
                        P1[q].ap()[prow:prow + (F // 2) * 128, :]
                        .rearrange("(hi p) f -> p hi f", hi=F // 2), st1[0][:])
            def rs1(q):
                nc.gpsimd.collective_compute(
                    "ReduceScatter", mybir.AluOpType.add,
                    replica_groups=groups,
                    ins=[P1[q].ap()], outs=[H1[q].ap()])
            scatter_layer(t1, c.HID, emit1, qhook=rs1)

        # Phase B2: per local block: relu(h1) -> transpose -> @W2 -> t2
        # (loads/stores batched 4 blocks per DMA within each quarter)
        with tc.tile_pool(name="ph1", bufs=3) as ph1, \
             tc.tile_pool(name="pds", bufs=3) as pds, \
             tc.tile_pool(name="pst", bufs=2, space="PSUM") as pst, \
             tc.tile_pool(name="psd", bufs=2, space="PSUM") as psd:
            for q in range(NQ):
                for lb0 in range(0, c.QB[q], 4):
                    gn = min(4, c.QB[q] - lb0)
                    nhi = gn // 2
                    hb = ph1.tile((128, 2, 256), f16, tag="hb", name="hb")
                    nc.sync.dma_start(
                        hb[:, 0:nhi, :],
                        H1[q].ap()[(lb0 // 2) * 128:(lb0 // 2 + nhi) * 128, :]
                        .rearrange("(hi p) f -> p hi f", hi=nhi))
                    s2 = pds.tile((128, 4, 128), f16, name='s2')
                    h1b4 = ph1.tile((128, 2, 256), f16, tag="h1b",
                                    name="h1b")
                    nc.scalar.activation(h1b4[:, 0:nhi, :],
                                         hb[:, 0:nhi, :], relu)
                    for i in range(gn):
                        tp = pst.tile((c.HID, 128), f16, space="PSUM")
                        nc.tensor.transpose(
                            out=tp[:],
                            in_=h1b4[:, i // 2,
                                     (i % 2) * 128:(i % 2 + 1) * 128],
                            identity=ident_sb[:])
                        h1t = ph1.tile((c.HID, 128), f16, tag="h1t",
                                       name="h1t")
                        nc.vector.tensor_copy(h1t[:], tp[:])
                        ps2 = psd.tile((128, 128), f32, space="PSUM")
                        nc.tensor.matmul(ps2[:], h1t[:], w2_sb[:],
                                         start=True, stop=True)
                        if i % 2 == 0:
                            nc.scalar.activation(s2[:, i, :], ps2[:], copyf)
                        else:
                            nc.vector.tensor_copy(s2[:, i, :], ps2[:])
                    b0 = c.QSTART[q] + lb0
                    nc.sync.dma_start(
                        t2.ap()[b0 * 128:(b0 + gn) * 128, :]
                        .rearrange("(blk p) f -> p blk f", blk=gn),
                        s2[:, 0:gn, :])

        # Layer-2 scatter -> P2 partials (quad/pair-packed rows)
        with tc.tile_pool(name="pe2", bufs=4) as pe2:
            st2 = [None]

            def emit2(p, ps):
                q, roff = pmeta(p)
                pk = pk2[q]
                i = p % pk
                if i == 0:
                    st2[0] = pe2.tile((128, 256), f16, name='st2')
                nc.scalar.activation(
                    st2[0][:, i * c.OUT:(i + 1) * c.OUT], ps[:], copyf)
                if i == pk - 1:
                    grow = (roff // pk) - (pk - 1) * 128 // pk
                    nc.sync.dma_start(
                        P2[q].ap()[grow:grow + 128, :],
                        st2[0][:, 0:pk * c.OUT])
            def rs2(q):
                nc.gpsimd.collective_compute(
                    "ReduceScatter", mybir.AluOpType.add,
                    replica_groups=groups,
                    ins=[P2[q].ap()], outs=[O2[q].ap()])
            scatter_layer(t2, c.OUT, emit2, qhook=rs2)

        # Final: O2 -> fp32 out (packed loads; last block is short)
        with tc.tile_pool(name="po", bufs=3) as po:
            for q in range(NQ):
                pk = pk2[q]
                for lb0 in range(0, c.QB[q], pk):
                    gn = pk
                    ot = po.tile((128, 4, c.OUT), f16, tag="ot", name="ot")
                    nc.sync.dma_start(
                        ot[:, 0:gn, :].rearrange("p blk f -> p (blk f)"),
                        O2[q].ap()[(lb0 // pk) * 128:
                                   (lb0 // pk + 1) * 128, :])
                    of = po.tile((128, 4, c.OUT), f32, tag="of", name="of")
                    nc.scalar.activation(of[:, 0:gn, :], ot[:, 0:gn, :], copyf)
                    b0 = c.QSTART[q] + lb0
                    full = min(gn, max(0, (c.NPC // 128) - b0))
                    if full > 0:
                        nc.sync.dma_start(
                            out.ap()[b0 * 128:(b0 + full) * 128, :]
                            .rearrange("(blk p) f -> p blk f", blk=full),
                            of[:, 0:full, :])
                    for i in range(full, gn):
                        b = b0 + i
                        rows = min(128, c.NPC - b * 128)
                        if rows > 0:
                            nc.sync.dma_start(
                                out.ap()[b * 128:b * 128 + rows, :],
                                of[0:rows, i, :])

    nc.compile()
    return nc


def make_inputs(cfg, features, edge_index, W1, W2):
    c = cfg
    colidx, rowloc, chunks, qoff, nch, cw, cnt128 = prep_edges(
        cfg, edge_index)
    iota2d = np.broadcast_to(np.arange(128, dtype=np.float16),
                             (128, 128)).copy()
    ident = np.eye(128, dtype=np.float16)
    w1 = np.ascontiguousarray(np.asarray(W1, np.float16))
    w2pad = np.zeros((c.HID, 128), np.float16)
    w2pad[:, :c.OUT] = np.asarray(W2, np.float16)
    in_maps = []
    for cc in range(c.NCORE):
        xc = np.asarray(features[cc * c.NPC:(cc + 1) * c.NPC], np.float32)
        xt = np.zeros((c.IN, c.NPP), np.float16)
        xt[:, :c.NPC] = xc.T.astype(np.float16)
        in_maps.append({
            "xT": np.ascontiguousarray(xt),
            "w1": w1, "w2p": w2pad, "iota": iota2d, "ident": ident,
            "colidx": np.ascontiguousarray(colidx[cc]),
            "rowloc": np.ascontiguousarray(rowloc[cc]),
        })
    return in_maps, chunks, qoff, nch, cw, cnt128


_LAST_NC = None


def kernel(features, edge_index, W1, W2):
    global _LAST_NC
    cfg = CFG
    in_maps, chunks, qoff, nch, cw, cnt128 = make_inputs(
        cfg, features, edge_index, W1, W2)
    nc = build(cfg, chunks, qoff, nch, cw, cnt128)
    _LAST_NC = nc
    res = bass_utils.run_bass_kernel_spmd(
        nc, in_maps, core_ids=list(range(cfg.NCORE)))
    return np.concatenate(
        [res.results[cc]["out"] for cc in range(cfg.NCORE)], axis=0)



# revision 5
# speedup vs baseline: 1.0100x; 1.0007x over previous
"""2-layer GCN (gnn_message_passing) on 8 Trainium2 NeuronCores — v3.

Source-sharded: each core owns 12500 nodes (features + support rows local).
Per layer: support = X_c @ W (local, PE) -> local HBM table -> per GLOBAL dest
block: dma_gather local source rows (edges bucketed by dest block on host,
int16 local indices), scatter into the block via one-hot matmul in PSUM ->
partial-output tables (per dest quarter) -> chunked ReduceScatter(add) sums
the 8 cores' partials; each core receives its own 12500 rows. ReLU + W2
transform after RS1; layer-2 scatter reuses the same edge buffers (same edge
order) with width 64. Collectives are out-small (RS) and overlap the scatter
pipeline via per-quarter tensors.
"""
import sys
sys.path.insert(0, "/opt/trn_rl_repo")

import numpy as np
from contextlib import ExitStack

import concourse.bass as bass
import concourse.bacc as bacc
import concourse.tile as tile
from concourse import bass_utils
from concourse import mybir
from concourse.library_config import mlp

PADVAL = 200.0
GMAX = 8   # max 128-idx chunks per dma_gather call (HW limit: 1024 idx)
F = 4      # dest blocks per supergroup
NQ = 7     # ReduceScatter chunks (dest sevenths; 14 blocks each, even for pairing)


class Config:
    def __init__(self, n=100000, in_dim=256, hid=128, out_dim=64, ncore=8):
        self.N = n
        self.IN = in_dim
        self.HID = hid
        self.OUT = out_dim
        self.NCORE = ncore
        self.NPC = n // ncore
        assert self.NPC * ncore == n
        self.NB = (self.NPC + 127) // 128          # 98 local blocks
        self.NPP = self.NB * 128                   # 12544
        self.GB = ncore * self.NB                  # 784 global dest blocks
        self.KT = in_dim // 128
        # quarter sizes in local blocks: quad-packable regions + runt
        self.QB = [16] * 6 + [2]
        assert sum(self.QB) == self.NB and len(self.QB) == NQ
        self.QSTART = [sum(self.QB[:q]) for q in range(NQ)]
        assert self.NPP <= 32767  # int16 gather idx


CFG = Config()


def prep_edges(cfg, edge_index):
    """Bucket each core's SOURCE-owned edges by global dest block in the
    quarter-major P-table order. Returns per-core colidx (16-wrapped int16
    local source row), rowloc (fp16 dest-row-in-block), uniform chunk counts
    per P-block, and chunk offsets."""
    c = cfg
    row = np.asarray(edge_index[0], dtype=np.int64)   # dest
    col = np.asarray(edge_index[1], dtype=np.int64)   # src
    score = col // c.NPC                              # owner core (source)
    sloc = (col - score * c.NPC).astype(np.int16)     # gather row in t-table
    dcore = row // c.NPC
    dl = row - dcore * c.NPC
    db = dl // 128                                    # dest local block
    rib = (dl % 128).astype(np.float16)
    # quarter of dest block
    qid = np.searchsorted(np.asarray(c.QSTART + [c.NB]), db, side="right") - 1
    # P-block index: quarter-major, core-major inside
    qb = np.asarray(c.QB)[qid]
    blocks_before = np.asarray([sum(c.QB[:q]) for q in range(NQ)])[qid] * c.NCORE
    pblk = blocks_before + dcore * qb + (db - np.asarray(c.QSTART)[qid])

    key = score * c.GB + pblk
    order = np.argsort(key, kind="stable")
    key_s = key[order]
    sloc_s = sloc[order]
    rib_s = rib[order]
    counts = np.bincount(key_s, minlength=c.NCORE * c.GB).reshape(
        c.NCORE, c.GB)
    starts = np.concatenate([[0], np.cumsum(counts.reshape(-1))])

    chunks = -(-counts.max(axis=0) // 128)            # (GB,) uniform chunks
    nch = int(chunks.sum())
    cw = nch * 8
    qoff = np.concatenate([[0], np.cumsum(chunks)])   # chunk offset per block

    colidx = np.zeros((c.NCORE, 128, cw), np.int16)
    rowloc = np.full((c.NCORE, 128, nch), PADVAL, np.float16)
    for cc in range(c.NCORE):
        for p in range(c.GB):
            nk = int(chunks[p])
            if nk == 0:
                continue
            ki = cc * c.GB + p
            s, e = starts[ki], starts[ki + 1]
            cnt = e - s
            cap = nk * 128
            tl = np.zeros(cap, np.int16)
            tl[:cnt] = sloc_s[s:e]
            rb = np.full(cap, PADVAL, np.float16)
            rb[:cnt] = rib_s[s:e]
            qo = int(qoff[p])
            colidx[cc][:, qo * 8:(qo + nk) * 8] = np.tile(
                tl.reshape(-1, 16).T, (8, 1))
            rowloc[cc][:, qo:qo + nk] = rb.reshape(nk, 128).T
    cnt128 = counts.max(axis=0)                       # (GB,) true max counts
    return colidx, rowloc, chunks, qoff, nch, cw, cnt128


def build(cfg, chunks, qoff, nch, cw, cnt128):
    c = cfg
    nc = bacc.Bacc(None, target_bir_lowering=False, debug=False,
                   num_devices=c.NCORE, name="gcnv3", num_swdge_queues=4)
    f16, f32, i16 = mybir.dt.float16, mybir.dt.float32, mybir.dt.int16
    f8 = mybir.dt.float8e4
    relu = mybir.ActivationFunctionType.Relu
    copyf = mybir.ActivationFunctionType.Copy

    xT = nc.dram_tensor("xT", (c.IN, c.NPP), f16, kind="ExternalInput")
    w1 = nc.dram_tensor("w1", (c.IN, c.HID), f16, kind="ExternalInput")
    w2p = nc.dram_tensor("w2p", (c.HID, 128), f16, kind="ExternalInput")
    iota = nc.dram_tensor("iota", (128, 128), f16, kind="ExternalInput")
    iotar = nc.dram_tensor("iotar", (128, 128 * 8), f16, kind="ExternalInput")
    ident = nc.dram_tensor("ident", (128, 128), f16, kind="ExternalInput")
    colidx = nc.dram_tensor("colidx", (128, cw), i16, kind="ExternalInput")
    rowloc = nc.dram_tensor("rowloc", (128, nch), f16, kind="ExternalInput")
    out = nc.dram_tensor("out", (c.NPC, c.OUT), f32, kind="ExternalOutput")

    t1 = nc.dram_tensor("t1", (c.NPP, c.HID), f16, kind="Internal")
    t2 = nc.dram_tensor("t2", (c.NPP, 128), f16, kind="Internal")
    # per-quarter partial tables + RS outputs (separate tensors so each RS
    # only waits on its own quarter's writers)
    # L1 partials pack two dest blocks per row (512B rows -> full-rate DMA)
    P1 = [nc.dram_tensor(f"P1q{q}", (c.NCORE * (c.QB[q] // 2) * 128, 256), f16,
                         kind="Internal") for q in range(NQ)]
    H1 = [nc.dram_tensor(f"H1q{q}", ((c.QB[q] // 2) * 128, 256), f16,
                         kind="Internal") for q in range(NQ)]
    # L2 partials pack 4 (or 2, runt quarter) dest blocks per row
    pk2 = [4 if c.QB[q] % 4 == 0 else 2 for q in range(NQ)]
    P2 = [nc.dram_tensor(f"P2q{q}",
                         (c.NCORE * (c.QB[q] // pk2[q]) * 128,
                          c.OUT * pk2[q]), f16,
                         kind="Internal") for q in range(NQ)]
    O2 = [nc.dram_tensor(f"O2q{q}",
                         ((c.QB[q] // pk2[q]) * 128, c.OUT * pk2[q]), f16,
                         kind="Internal") for q in range(NQ)]

    groups = [list(range(c.NCORE))]
    iseq = mybir.AluOpType.is_equal

    # P-block -> (quarter, core, block-in-quarter) and row offset in P[q]
    def pmeta(p):
        acc = 0
        for q in range(NQ):
            nb = c.NCORE * c.QB[q]
            if p < acc + nb:
                j = p - acc
                return q, j * 128  # row offset within P[q]
            acc += nb
        raise AssertionError

    with ExitStack() as ctx:
        tc = ctx.enter_context(tile.TileContext(nc))
        nc.gpsimd.load_library(mlp)
        cpool = ctx.enter_context(tc.tile_pool(name="const", bufs=1))
        # shared gather pool, warmed once (hidden under phase A): trimmed
        # calls leave tail rows unwritten, which must be stale-finite
        pg = ctx.enter_context(tc.tile_pool(name="pg", bufs=24))
        for _ in range(24):
            wt = pg.tile((128, GMAX, 128), mybir.dt.float16, name="gt")
            nc.vector.memset(wt[:], 0.0)
        iota_sb = cpool.tile((128, 128), f16, tag="iota")
        nc.sync.dma_start(iota_sb[:], iota[:])
        iotar_sb = cpool.tile((128, 128, 8), f16, tag="iotar")
        nc.sync.dma_start(iotar_sb[:].rearrange("p r c -> p (r c)"), iotar[:])
        ident_sb = cpool.tile((128, 128), f16, tag="ident")
        nc.sync.dma_start(ident_sb[:], ident[:])
        w2_sb = cpool.tile((c.HID, 128), f16, tag="w2")
        nc.sync.dma_start(w2_sb[:], w2p[:])

        # Phase A: t1 = X_c @ W1 (local support)
        with tc.tile_pool(name="pa", bufs=1) as pa, \
             tc.tile_pool(name="pas", bufs=3) as pas, \
             tc.tile_pool(name="psa", bufs=2, space="PSUM") as psa:
            xk, w1k = [], []
            half = (c.NB // 2) * 128
            for k in range(c.KT):
                t = pa.tile((128, c.HID), f16, tag=f"w{k}", name=f"w1k{k}")
                nc.sync.dma_start(t[:], w1.ap()[k * 128:(k + 1) * 128, :])
                w1k.append(t)
            for k in range(c.KT):
                # column halves: block matmuls start at 50% of the x load
                ta = pa.tile((128, half), f16, tag=f"xa{k}", name=f"xka{k}")
                nc.sync.dma_start(ta[:],
                                  xT.ap()[k * 128:(k + 1) * 128, 0:half])
                tb = pa.tile((128, c.NPP - half), f16, tag=f"xb{k}",
                             name=f"xkb{k}")
                nc.sync.dma_start(tb[:],
                                  xT.ap()[k * 128:(k + 1) * 128, half:])
                xk.append((ta, tb))
            for b0 in range(0, c.NB, 2):
                gn = min(2, c.NB - b0)
                s1 = pas.tile((128, 2, c.HID), f16, name='s1')
                for i in range(gn):
                    b = b0 + i
                    ps = psa.tile((128, c.HID), f32, space="PSUM")
                    for k in range(c.KT):
                        xt_, off = ((xk[k][0], 0) if (b + 1) * 128 <= half
                                    else (xk[k][1], half))
                        nc.tensor.matmul(
                            ps[:],
                            xt_[:, b * 128 - off:(b + 1) * 128 - off],
                            w1k[k][:], start=(k == 0), stop=(k == c.KT - 1))
                    nc.scalar.activation(s1[:, i, :], ps[:], copyf)
                nc.sync.dma_start(
                    t1.ap()[b0 * 128:(b0 + gn) * 128, :]
                    .rearrange("(blk p) f -> p blk f", blk=gn),
                    s1[:, 0:gn, :])

        # edge metadata loads queued after phase-A inputs (same SP DMA FIFO)
        rowloc_sb = cpool.tile((128, nch), f16, tag="rowloc")
        nc.sync.dma_start(rowloc_sb[:], rowloc[:])
        colidx_sb = cpool.tile((128, cw), i16, tag="colidx")
        nc.sync.dma_start(colidx_sb[:], colidx[:])

        # chunk index -> trailing-pad trim when a gather call ends exactly at
        # this chunk boundary (a cell's last chunk): skip the cell's pad
        # slots.  num_idxs need not be a multiple of 128; untouched tail rows
        # of the (reused, uniform-shape) pool buffers hold stale-but-finite
        # data that the zero one-hot columns nullify.  The first 24 calls per
        # layer stay untrimmed so every pool buffer is fully written once.
        endtrim = {}
        for b in range(c.GB):
            if chunks[b] > 0:
                endtrim[int(qoff[b + 1])] = int(chunks[b] * 128 - cnt128[b])

        # P-block index at which each quarter region ends
        qends = {}
        acc = 0
        for q in range(NQ):
            acc += c.NCORE * c.QB[q]
            qends[acc] = q

        def scatter_layer(tab, width, emit, qhook=None):
            """Gather+scatter all P-blocks from local table `tab`;
            emit(p, psum_ap) per finished block; qhook(q) fires inline after
            the last supergroup of quarter q so collectives dispatch from the
            Pool sequencer mid-layer instead of queuing behind all gathers."""
            gq = [0]
            with tc.tile_pool(name="poh", bufs=3) as poh, \
                 tc.tile_pool(name="pso", bufs=2, space="PSUM") as pso:
                for g0 in range(0, c.GB, F):
                    bs = list(range(g0, min(g0 + F, c.GB)))
                    wg = int(chunks[bs].sum())
                    qg0 = int(qoff[bs[0]])
                    cblk = [b for b in bs for _ in range(int(chunks[b]))]
                    first, last = {}, {}
                    for ci, b in enumerate(cblk):
                        first.setdefault(b, ci)
                        last[b] = ci
                    psb = {b: pso.tile((128, width), f32, space="PSUM",
                                       tag=f"ps{b - g0}", name=f"ps{b - g0}")
                           for b in bs}
                    for b in bs:
                        if b not in first:
                            zt = pg.tile((128, width), f16)
                            nc.vector.memset(zt[:], 0.0)
                            nc.tensor.matmul(psb[b][:], ident_sb[:, 0:width],
                                             zt[:], start=True, stop=True)
                    if wg > 0:
                        oh = poh.tile((128, wg, 128), f16)
                        nc.vector.tensor_tensor(
                            out=oh[:],
                            in0=rowloc_sb[:, qg0:qg0 + wg].unsqueeze(2)
                                .to_broadcast((128, wg, 128)),
                            in1=iota_sb[:].unsqueeze(1)
                                .to_broadcast((128, wg, 128)),
                            op=iseq)
                        # choose call windows (<= GMAX chunks, minimal
                        # count) maximizing endings on cell boundaries so
                        # their trailing pads can be trimmed
                        K = -(-wg // GMAX)
                        dpk = {0: (0, [])}
                        ends = None
                        for _ in range(K):
                            nxt = {}
                            for pos, (t, es) in dpk.items():
                                for e in range(pos + 1,
                                               min(pos + GMAX, wg) + 1):
                                    tt = t + (1 if (qg0 + e) in endtrim
                                              else 0)
                                    if e not in nxt or nxt[e][0] < tt:
                                        nxt[e] = (tt, es + [e])
                            dpk = nxt
                            if wg in dpk:
                                ends = dpk[wg][1]
                                break
                        assert ends is not None
                        for wi, e in enumerate(ends):
                            s0 = ends[wi - 1] if wi else 0
                            sn = e - s0
                            trim = endtrim.get(qg0 + e, 0)
                            nidx = sn * 128 - trim
                            gt = pg.tile((128, GMAX, 128), f16, name="gt")
                            nc.gpsimd.dma_gather(
                                gt[:, 0:sn, :], tab.ap(),
                                colidx_sb[:, (qg0 + s0) * 8:
                                          (qg0 + s0 + sn) * 8],
                                nidx, nidx, 128,
                                queue_num=gq[0] % 4)
                            gq[0] += 1
                            for j in range(sn):
                                ci = s0 + j
                                b = cblk[ci]
                                nc.tensor.matmul(
                                    psb[b][:], oh[:, ci, :],
                                    gt[:, j, 0:width],
                                    start=(ci == first[b]),
                                    stop=(ci == last[b]))
                    for b in bs:
                        emit(b, psb[b])
                    if qhook is not None and (g0 + F) in qends:
                        qhook(qends[g0 + F])

        # Phase B2: per local block: relu(h1) -> transpose -> @W2 -> t2
        # (loads/stores batched 4 blocks per DMA; emitted per quarter from
        # the RS1 hook so the work hides inside the L1 scatter phase)
        def phase_b2_quarter(q, ph1, pds, pst, psd):
            if True:
                for lb0 in range(0, c.QB[q], 4):
                    gn = min(4, c.QB[q] - lb0)
                    nhi = gn // 2
                    hb = ph1.tile((128, 2, 256), f16, tag="hb", name="hb")
                    nc.sync.dma_start(
                        hb[:, 0:nhi, :],
                        H1[q].ap()[(lb0 // 2) * 128:(lb0 // 2 + nhi) * 128, :]
                        .rearrange("(hi p) f -> p hi f", hi=nhi))
                    s2 = pds.tile((128, 4, 128), f16, name='s2')
                    h1b4 = ph1.tile((128, 2, 256), f16, tag="h1b",
                                    name="h1b")
                    nc.scalar.activation(h1b4[:, 0:nhi, :],
                                         hb[:, 0:nhi, :], relu)
                    for i in range(gn):
                        tp = pst.tile((c.HID, 128), f16, space="PSUM")
                        nc.tensor.transpose(
                            out=tp[:],
                            in_=h1b4[:, i // 2,
                                     (i % 2) * 128:(i % 2 + 1) * 128],
                            identity=ident_sb[:])
                        h1t = ph1.tile((c.HID, 128), f16, tag="h1t",
                                       name="h1t")
                        nc.vector.tensor_copy(h1t[:], tp[:])
                        ps2 = psd.tile((128, 128), f32, space="PSUM")
                        nc.tensor.matmul(ps2[:], h1t[:], w2_sb[:],
                                         start=True, stop=True)
                        if i % 2 == 0:
                            nc.scalar.activation(s2[:, i, :], ps2[:], copyf)
                        else:
                            nc.vector.tensor_copy(s2[:, i, :], ps2[:])
                    b0 = c.QSTART[q] + lb0
                    nc.sync.dma_start(
                        t2.ap()[b0 * 128:(b0 + gn) * 128, :]
                        .rearrange("(blk p) f -> p blk f", blk=gn),
                        s2[:, 0:gn, :])

        # Final: O2 -> fp32 out per quarter (from the RS2 hook)
        def final_quarter(q, po):
            if True:
                pk = pk2[q]
                for lb0 in range(0, c.QB[q], pk):
                    gn = pk
                    ot = po.tile((128, 4, c.OUT), f16, tag="ot", name="ot")
                    nc.sync.dma_start(
                        ot[:, 0:gn, :].rearrange("p blk f -> p (blk f)"),
                        O2[q].ap()[(lb0 // pk) * 128:
                                   (lb0 // pk + 1) * 128, :])
                    of = po.tile((128, 4, c.OUT), f32, tag="of", name="of")
                    nc.scalar.activation(of[:, 0:gn, :], ot[:, 0:gn, :], copyf)
                    b0 = c.QSTART[q] + lb0
                    full = min(gn, max(0, (c.NPC // 128) - b0))
                    if full > 0:
                        nc.sync.dma_start(
                            out.ap()[b0 * 128:(b0 + full) * 128, :]
                            .rearrange("(blk p) f -> p blk f", blk=full),
                            of[:, 0:full, :])
                    for i in range(full, gn):
                        b = b0 + i
                        rows = min(128, c.NPC - b * 128)
                        if rows > 0:
                            nc.sync.dma_start(
                                out.ap()[b * 128:b * 128 + rows, :],
                                of[0:rows, i, :])

        # Layer-1 scatter -> P1 partials (pair-packed, batched per supergroup)
        with tc.tile_pool(name="pe1", bufs=4) as pe1:
            st1 = [None]

            def emit1(p, ps):
                i = p % F
                if i == 0:
                    st1[0] = pe1.tile((128, F // 2, 256), f16, name='st1')
                nc.scalar.activation(
                    st1[0][:, i // 2, (i % 2) * 128:(i % 2 + 1) * 128],
                    ps[:], copyf)
                if i == F - 1:
                    q, roff = pmeta(p - F + 1)
                    prow = roff // 2
                    nc.sync.dma_start(
                        P1[q].ap()[prow:prow + (F // 2) * 128, :]
                        .rearrange("(hi p) f -> p hi f", hi=F // 2), st1[0][:])
            def rs1(q):
                nc.gpsimd.collective_compute(
                    "ReduceScatter", mybir.AluOpType.add,
                    replica_groups=groups,
                    ins=[P1[q].ap()], outs=[H1[q].ap()])
            scatter_layer(t1, c.HID, emit1, qhook=rs1)

        with tc.tile_pool(name="ph1", bufs=3) as ph1, \
             tc.tile_pool(name="pds", bufs=3) as pds, \
             tc.tile_pool(name="pst", bufs=2, space="PSUM") as pst, \
             tc.tile_pool(name="psd", bufs=2, space="PSUM") as psd:
            for q in range(NQ):
                phase_b2_quarter(q, ph1, pds, pst, psd)

        # Layer-2 scatter -> P2 partials (quad/pair-packed rows)
        with tc.tile_pool(name="pe2", bufs=4) as pe2, \
             tc.tile_pool(name="po", bufs=3) as po:
            st2 = [None]

            def emit2(p, ps):
                q, roff = pmeta(p)
                pk = pk2[q]
                i = p % pk
                if i == 0:
                    st2[0] = pe2.tile((128, 256), f16, name='st2')
                nc.scalar.activation(
                    st2[0][:, i * c.OUT:(i + 1) * c.OUT], ps[:], copyf)
                if i == pk - 1:
                    grow = (roff // pk) - (pk - 1) * 128 // pk
                    nc.sync.dma_start(
                        P2[q].ap()[grow:grow + 128, :],
                        st2[0][:, 0:pk * c.OUT])
            def rs2(q):
                nc.gpsimd.collective_compute(
                    "ReduceScatter", mybir.AluOpType.add,
                    replica_groups=groups,
                    ins=[P2[q].ap()], outs=[O2[q].ap()])
                final_quarter(q, po)
            scatter_layer(t2, c.OUT, emit2, qhook=rs2)

    nc.compile()
    return nc


def make_inputs(cfg, features, edge_index, W1, W2):
    c = cfg
    colidx, rowloc, chunks, qoff, nch, cw, cnt128 = prep_edges(
        cfg, edge_index)
    iota2d = np.broadcast_to(np.arange(128, dtype=np.float16),
                             (128, 128)).copy()
    iotar = np.broadcast_to(np.arange(128, dtype=np.float16)[:, None],
                            (128, 8)).reshape(1, 1024)
    iotar = np.broadcast_to(iotar, (128, 1024)).copy()
    ident = np.eye(128, dtype=np.float16)
    w1 = np.ascontiguousarray(np.asarray(W1, np.float16))
    w2pad = np.zeros((c.HID, 128), np.float16)
    w2pad[:, :c.OUT] = np.asarray(W2, np.float16)
    in_maps = []
    for cc in range(c.NCORE):
        xc = np.asarray(features[cc * c.NPC:(cc + 1) * c.NPC], np.float32)
        xt = np.zeros((c.IN, c.NPP), np.float16)
        xt[:, :c.NPC] = xc.T.astype(np.float16)
        in_maps.append({
            "xT": np.ascontiguousarray(xt),
            "w1": w1, "w2p": w2pad, "iota": iota2d, "iotar": iotar,
            "ident": ident,
            "colidx": np.ascontiguousarray(colidx[cc]),
            "rowloc": np.ascontiguousarray(rowloc[cc]),
        })
    return in_maps, chunks, qoff, nch, cw, cnt128


_LAST_NC = None


def kernel(features, edge_index, W1, W2):
    global _LAST_NC
    cfg = CFG
    in_maps, chunks, qoff, nch, cw, cnt128 = make_inputs(
        cfg, features, edge_index, W1, W2)
    nc = build(cfg, chunks, qoff, nch, cw, cnt128)
    _LAST_NC = nc
    res = bass_utils.run_bass_kernel_spmd(
        nc, in_maps, core_ids=list(range(cfg.NCORE)))
    return np.concatenate(
        [res.results[cc]["out"] for cc in range(cfg.NCORE)], axis=0)



# revision 6
# speedup vs baseline: 1.0116x; 1.0015x over previous
"""2-layer GCN (gnn_message_passing) on 8 Trainium2 NeuronCores — v3.

Source-sharded: each core owns 12500 nodes (features + support rows local).
Per layer: support = X_c @ W (local, PE) -> local HBM table -> per GLOBAL dest
block: dma_gather local source rows (edges bucketed by dest block on host,
int16 local indices), scatter into the block via one-hot matmul in PSUM ->
partial-output tables (per dest quarter) -> chunked ReduceScatter(add) sums
the 8 cores' partials; each core receives its own 12500 rows. ReLU + W2
transform after RS1; layer-2 scatter reuses the same edge buffers (same edge
order) with width 64. Collectives are out-small (RS) and overlap the scatter
pipeline via per-quarter tensors.
"""
import sys
sys.path.insert(0, "/opt/trn_rl_repo")

import numpy as np
from contextlib import ExitStack

import concourse.bass as bass
import concourse.bacc as bacc
import concourse.tile as tile
from concourse import bass_utils
from concourse import mybir
from concourse.library_config import mlp

PADVAL = 200.0
GMAX = 8   # max 128-idx chunks per dma_gather call (HW limit: 1024 idx)
F = 4      # dest blocks per supergroup
NQ = 7     # ReduceScatter chunks (dest sevenths; 14 blocks each, even for pairing)


class Config:
    def __init__(self, n=100000, in_dim=256, hid=128, out_dim=64, ncore=8):
        self.N = n
        self.IN = in_dim
        self.HID = hid
        self.OUT = out_dim
        self.NCORE = ncore
        self.NPC = n // ncore
        assert self.NPC * ncore == n
        self.NB = (self.NPC + 127) // 128          # 98 local blocks
        self.NPP = self.NB * 128                   # 12544
        self.GB = ncore * self.NB                  # 784 global dest blocks
        self.KT = in_dim // 128
        # quarter sizes in local blocks: quad-packable regions + runt
        self.QB = [16] * 6 + [2]
        assert sum(self.QB) == self.NB and len(self.QB) == NQ
        self.QSTART = [sum(self.QB[:q]) for q in range(NQ)]
        assert self.NPP <= 32767  # int16 gather idx


CFG = Config()


def prep_edges(cfg, edge_index):
    """Bucket each core's SOURCE-owned edges by global dest block in the
    quarter-major P-table order. Returns per-core colidx (16-wrapped int16
    local source row), rowloc (fp16 dest-row-in-block), uniform chunk counts
    per P-block, and chunk offsets."""
    c = cfg
    row = np.asarray(edge_index[0], dtype=np.int64)   # dest
    col = np.asarray(edge_index[1], dtype=np.int64)   # src
    score = col // c.NPC                              # owner core (source)
    sloc = (col - score * c.NPC).astype(np.int16)     # gather row in t-table
    dcore = row // c.NPC
    dl = row - dcore * c.NPC
    db = dl // 128                                    # dest local block
    rib = (dl % 128).astype(np.float16)
    # quarter of dest block
    qid = np.searchsorted(np.asarray(c.QSTART + [c.NB]), db, side="right") - 1
    # P-block index: quarter-major, core-major inside
    qb = np.asarray(c.QB)[qid]
    blocks_before = np.asarray([sum(c.QB[:q]) for q in range(NQ)])[qid] * c.NCORE
    pblk = blocks_before + dcore * qb + (db - np.asarray(c.QSTART)[qid])

    key = score * c.GB + pblk
    order = np.argsort(key, kind="stable")
    key_s = key[order]
    sloc_s = sloc[order]
    rib_s = rib[order]
    counts = np.bincount(key_s, minlength=c.NCORE * c.GB).reshape(
        c.NCORE, c.GB)
    starts = np.concatenate([[0], np.cumsum(counts.reshape(-1))])

    chunks = -(-counts.max(axis=0) // 128)            # (GB,) uniform chunks
    nch = int(chunks.sum())
    cw = nch * 8
    qoff = np.concatenate([[0], np.cumsum(chunks)])   # chunk offset per block

    colidx = np.zeros((c.NCORE, 128, cw), np.int16)
    rowloc = np.full((c.NCORE, 128, nch), PADVAL, np.float16)
    for cc in range(c.NCORE):
        for p in range(c.GB):
            nk = int(chunks[p])
            if nk == 0:
                continue
            ki = cc * c.GB + p
            s, e = starts[ki], starts[ki + 1]
            cnt = e - s
            cap = nk * 128
            tl = np.zeros(cap, np.int16)
            tl[:cnt] = sloc_s[s:e]
            rb = np.full(cap, PADVAL, np.float16)
            rb[:cnt] = rib_s[s:e]
            qo = int(qoff[p])
            colidx[cc][:, qo * 8:(qo + nk) * 8] = np.tile(
                tl.reshape(-1, 16).T, (8, 1))
            rowloc[cc][:, qo:qo + nk] = rb.reshape(nk, 128).T
    cnt128 = counts.max(axis=0)                       # (GB,) true max counts
    return colidx, rowloc, chunks, qoff, nch, cw, cnt128


def build(cfg, chunks, qoff, nch, cw, cnt128):
    c = cfg
    nc = bacc.Bacc(None, target_bir_lowering=False, debug=False,
                   num_devices=c.NCORE, name="gcnv3", num_swdge_queues=4)
    f16, f32, i16 = mybir.dt.float16, mybir.dt.float32, mybir.dt.int16
    f8 = mybir.dt.float8e4
    relu = mybir.ActivationFunctionType.Relu
    copyf = mybir.ActivationFunctionType.Copy

    xT = nc.dram_tensor("xT", (c.IN, c.NPP), f16, kind="ExternalInput")
    w1 = nc.dram_tensor("w1", (c.IN, c.HID), f16, kind="ExternalInput")
    w2p = nc.dram_tensor("w2p", (c.HID, 128), f16, kind="ExternalInput")
    iota = nc.dram_tensor("iota", (128, 128), f16, kind="ExternalInput")
    iotar = nc.dram_tensor("iotar", (128, 128 * 8), f16, kind="ExternalInput")
    ident = nc.dram_tensor("ident", (128, 128), f16, kind="ExternalInput")
    colidx = nc.dram_tensor("colidx", (128, cw), i16, kind="ExternalInput")
    rowloc = nc.dram_tensor("rowloc", (128, nch), f16, kind="ExternalInput")
    out = nc.dram_tensor("out", (c.NPC, c.OUT), f32, kind="ExternalOutput")

    t1 = nc.dram_tensor("t1", (c.NPP, c.HID), f16, kind="Internal")
    t2 = nc.dram_tensor("t2", (c.NPP, 128), f16, kind="Internal")
    # per-quarter partial tables + RS outputs (separate tensors so each RS
    # only waits on its own quarter's writers)
    # L1 partials pack two dest blocks per row (512B rows -> full-rate DMA)
    P1 = [nc.dram_tensor(f"P1q{q}", (c.NCORE * (c.QB[q] // 2) * 128, 256), f16,
                         kind="Internal") for q in range(NQ)]
    H1 = [nc.dram_tensor(f"H1q{q}", ((c.QB[q] // 2) * 128, 256), f16,
                         kind="Internal") for q in range(NQ)]
    # L2 partials pack 4 (or 2, runt quarter) dest blocks per row
    pk2 = [4 if c.QB[q] % 4 == 0 else 2 for q in range(NQ)]
    P2 = [nc.dram_tensor(f"P2q{q}",
                         (c.NCORE * (c.QB[q] // pk2[q]) * 128,
                          c.OUT * pk2[q]), f16,
                         kind="Internal") for q in range(NQ)]
    O2 = [nc.dram_tensor(f"O2q{q}",
                         ((c.QB[q] // pk2[q]) * 128, c.OUT * pk2[q]), f16,
                         kind="Internal") for q in range(NQ)]

    groups = [list(range(c.NCORE))]
    iseq = mybir.AluOpType.is_equal

    # P-block -> (quarter, core, block-in-quarter) and row offset in P[q]
    def pmeta(p):
        acc = 0
        for q in range(NQ):
            nb = c.NCORE * c.QB[q]
            if p < acc + nb:
                j = p - acc
                return q, j * 128  # row offset within P[q]
            acc += nb
        raise AssertionError

    with ExitStack() as ctx:
        tc = ctx.enter_context(tile.TileContext(nc))
        nc.gpsimd.load_library(mlp)
        cpool = ctx.enter_context(tc.tile_pool(name="const", bufs=1))
        # shared gather pool, warmed once (hidden under phase A): trimmed
        # calls leave tail rows unwritten, which must be stale-finite
        pg = ctx.enter_context(tc.tile_pool(name="pg", bufs=24))
        for _ in range(24):
            wt = pg.tile((128, GMAX, 128), mybir.dt.float16, name="gt")
            nc.vector.memset(wt[:], 0.0)
        iota_sb = cpool.tile((128, 128), f16, tag="iota")
        nc.sync.dma_start(iota_sb[:], iota[:])
        iotar_sb = cpool.tile((128, 128, 8), f16, tag="iotar")
        nc.sync.dma_start(iotar_sb[:].rearrange("p r c -> p (r c)"), iotar[:])
        ident_sb = cpool.tile((128, 128), f16, tag="ident")
        nc.sync.dma_start(ident_sb[:], ident[:])
        w2_sb = cpool.tile((c.HID, 128), f16, tag="w2")
        nc.sync.dma_start(w2_sb[:], w2p[:])

        # Phase A: t1 = X_c @ W1 (local support)
        with tc.tile_pool(name="pa", bufs=1) as pa, \
             tc.tile_pool(name="pas", bufs=3) as pas, \
             tc.tile_pool(name="psa", bufs=2, space="PSUM") as psa:
            xk, w1k = [], []
            half = (c.NB // 2) * 128
            for k in range(c.KT):
                t = pa.tile((128, c.HID), f16, tag=f"w{k}", name=f"w1k{k}")
                nc.sync.dma_start(t[:], w1.ap()[k * 128:(k + 1) * 128, :])
                w1k.append(t)
            for k in range(c.KT):
                # column halves: block matmuls start at 50% of the x load
                ta = pa.tile((128, half), f16, tag=f"xa{k}", name=f"xka{k}")
                nc.sync.dma_start(ta[:],
                                  xT.ap()[k * 128:(k + 1) * 128, 0:half])
                tb = pa.tile((128, c.NPP - half), f16, tag=f"xb{k}",
                             name=f"xkb{k}")
                nc.sync.dma_start(tb[:],
                                  xT.ap()[k * 128:(k + 1) * 128, half:])
                xk.append((ta, tb))
            for b0 in range(0, c.NB, 2):
                gn = min(2, c.NB - b0)
                s1 = pas.tile((128, 2, c.HID), f16, name='s1')
                for i in range(gn):
                    b = b0 + i
                    ps = psa.tile((128, c.HID), f32, space="PSUM")
                    for k in range(c.KT):
                        xt_, off = ((xk[k][0], 0) if (b + 1) * 128 <= half
                                    else (xk[k][1], half))
                        nc.tensor.matmul(
                            ps[:],
                            xt_[:, b * 128 - off:(b + 1) * 128 - off],
                            w1k[k][:], start=(k == 0), stop=(k == c.KT - 1))
                    nc.scalar.activation(s1[:, i, :], ps[:], copyf)
                nc.sync.dma_start(
                    t1.ap()[b0 * 128:(b0 + gn) * 128, :]
                    .rearrange("(blk p) f -> p blk f", blk=gn),
                    s1[:, 0:gn, :])

        # edge metadata loads queued after phase-A inputs (same SP DMA FIFO)
        rowloc_sb = cpool.tile((128, nch), f16, tag="rowloc")
        nc.sync.dma_start(rowloc_sb[:], rowloc[:])
        colidx_sb = cpool.tile((128, cw), i16, tag="colidx")
        nc.sync.dma_start(colidx_sb[:], colidx[:])

        # chunk index -> trailing-pad trim when a gather call ends exactly at
        # this chunk boundary (a cell's last chunk): skip the cell's pad
        # slots.  num_idxs need not be a multiple of 128; untouched tail rows
        # of the (reused, uniform-shape) pool buffers hold stale-but-finite
        # data that the zero one-hot columns nullify.  The first 24 calls per
        # layer stay untrimmed so every pool buffer is fully written once.
        endtrim = {}
        for b in range(c.GB):
            if chunks[b] > 0:
                endtrim[int(qoff[b + 1])] = int(chunks[b] * 128 - cnt128[b])

        # P-block index at which each quarter region ends
        qends = {}
        acc = 0
        for q in range(NQ):
            acc += c.NCORE * c.QB[q]
            qends[acc] = q

        def scatter_layer(tab, width, emit, qhook=None):
            """Gather+scatter all P-blocks from local table `tab`;
            emit(p, psum_ap) per finished block; qhook(q) fires inline after
            the last supergroup of quarter q so collectives dispatch from the
            Pool sequencer mid-layer instead of queuing behind all gathers."""
            gq = [0]
            with tc.tile_pool(name="poh", bufs=3) as poh, \
                 tc.tile_pool(name="pso", bufs=2, space="PSUM") as pso:
                for g0 in range(0, c.GB, F):
                    bs = list(range(g0, min(g0 + F, c.GB)))
                    wg = int(chunks[bs].sum())
                    qg0 = int(qoff[bs[0]])
                    cblk = [b for b in bs for _ in range(int(chunks[b]))]
                    first, last = {}, {}
                    for ci, b in enumerate(cblk):
                        first.setdefault(b, ci)
                        last[b] = ci
                    psb = {b: pso.tile((128, width), f32, space="PSUM",
                                       tag=f"ps{b - g0}", name=f"ps{b - g0}")
                           for b in bs}
                    for b in bs:
                        if b not in first:
                            zt = pg.tile((128, width), f16)
                            nc.vector.memset(zt[:], 0.0)
                            nc.tensor.matmul(psb[b][:], ident_sb[:, 0:width],
                                             zt[:], start=True, stop=True)
                    if wg > 0:
                        oh = poh.tile((128, wg, 128), f16)
                        nc.vector.tensor_tensor(
                            out=oh[:],
                            in0=rowloc_sb[:, qg0:qg0 + wg].unsqueeze(2)
                                .to_broadcast((128, wg, 128)),
                            in1=iota_sb[:].unsqueeze(1)
                                .to_broadcast((128, wg, 128)),
                            op=iseq)
                        # choose call windows (<= GMAX chunks, minimal
                        # count) maximizing endings on cell boundaries so
                        # their trailing pads can be trimmed
                        K = -(-wg // GMAX)
                        dpk = {0: (0, [])}
                        ends = None
                        for _ in range(K):
                            nxt = {}
                            for pos, (t, es) in dpk.items():
                                for e in range(pos + 1,
                                               min(pos + GMAX, wg) + 1):
                                    tt = t + endtrim.get(qg0 + e, 0)
                                    if e not in nxt or nxt[e][0] < tt:
                                        nxt[e] = (tt, es + [e])
                            dpk = nxt
                            if wg in dpk:
                                ends = dpk[wg][1]
                                break
                        assert ends is not None
                        for wi, e in enumerate(ends):
                            s0 = ends[wi - 1] if wi else 0
                            sn = e - s0
                            trim = endtrim.get(qg0 + e, 0)
                            nidx = sn * 128 - trim
                            gt = pg.tile((128, GMAX, 128), f16, name="gt")
                            nc.gpsimd.dma_gather(
                                gt[:, 0:sn, :], tab.ap(),
                                colidx_sb[:, (qg0 + s0) * 8:
                                          (qg0 + s0 + sn) * 8],
                                nidx, nidx, 128,
                                queue_num=gq[0] % 4)
                            gq[0] += 1
                            for j in range(sn):
                                ci = s0 + j
                                b = cblk[ci]
                                nc.tensor.matmul(
                                    psb[b][:], oh[:, ci, :],
                                    gt[:, j, 0:width],
                                    start=(ci == first[b]),
                                    stop=(ci == last[b]))
                    for b in bs:
                        emit(b, psb[b])
                    if qhook is not None and (g0 + F) in qends:
                        qhook(qends[g0 + F])

        # Phase B2: per local block: relu(h1) -> transpose -> @W2 -> t2
        # (loads/stores batched 4 blocks per DMA; emitted per quarter from
        # the RS1 hook so the work hides inside the L1 scatter phase)
        def phase_b2_quarter(q, ph1, pds, pst, psd):
            if True:
                for lb0 in range(0, c.QB[q], 4):
                    gn = min(4, c.QB[q] - lb0)
                    nhi = gn // 2
                    hb = ph1.tile((128, 2, 256), f16, tag="hb", name="hb")
                    nc.sync.dma_start(
                        hb[:, 0:nhi, :],
                        H1[q].ap()[(lb0 // 2) * 128:(lb0 // 2 + nhi) * 128, :]
                        .rearrange("(hi p) f -> p hi f", hi=nhi))
                    s2 = pds.tile((128, 4, 128), f16, name='s2')
                    h1b4 = ph1.tile((128, 2, 256), f16, tag="h1b",
                                    name="h1b")
                    nc.scalar.activation(h1b4[:, 0:nhi, :],
                                         hb[:, 0:nhi, :], relu)
                    for i in range(gn):
                        tp = pst.tile((c.HID, 128), f16, space="PSUM")
                        nc.tensor.transpose(
                            out=tp[:],
                            in_=h1b4[:, i // 2,
                                     (i % 2) * 128:(i % 2 + 1) * 128],
                            identity=ident_sb[:])
                        h1t = ph1.tile((c.HID, 128), f16, tag="h1t",
                                       name="h1t")
                        nc.vector.tensor_copy(h1t[:], tp[:])
                        ps2 = psd.tile((128, 128), f32, space="PSUM")
                        nc.tensor.matmul(ps2[:], h1t[:], w2_sb[:],
                                         start=True, stop=True)
                        if i % 2 == 0:
                            nc.scalar.activation(s2[:, i, :], ps2[:], copyf)
                        else:
                            nc.vector.tensor_copy(s2[:, i, :], ps2[:])
                    b0 = c.QSTART[q] + lb0
                    nc.sync.dma_start(
                        t2.ap()[b0 * 128:(b0 + gn) * 128, :]
                        .rearrange("(blk p) f -> p blk f", blk=gn),
                        s2[:, 0:gn, :])

        # Final: O2 -> fp32 out per quarter (from the RS2 hook)
        def final_quarter(q, po):
            if True:
                pk = pk2[q]
                for lb0 in range(0, c.QB[q], pk):
                    gn = pk
                    ot = po.tile((128, 4, c.OUT), f16, tag="ot", name="ot")
                    nc.sync.dma_start(
                        ot[:, 0:gn, :].rearrange("p blk f -> p (blk f)"),
                        O2[q].ap()[(lb0 // pk) * 128:
                                   (lb0 // pk + 1) * 128, :])
                    of = po.tile((128, 4, c.OUT), f32, tag="of", name="of")
                    nc.scalar.activation(of[:, 0:gn, :], ot[:, 0:gn, :], copyf)
                    b0 = c.QSTART[q] + lb0
                    full = min(gn, max(0, (c.NPC // 128) - b0))
                    if full > 0:
                        nc.sync.dma_start(
                            out.ap()[b0 * 128:(b0 + full) * 128, :]
                            .rearrange("(blk p) f -> p blk f", blk=full),
                            of[:, 0:full, :])
                    for i in range(full, gn):
                        b = b0 + i
                        rows = min(128, c.NPC - b * 128)
                        if rows > 0:
                            nc.sync.dma_start(
                                out.ap()[b * 128:b * 128 + rows, :],
                                of[0:rows, i, :])

        # Layer-1 scatter -> P1 partials (pair-packed, batched per supergroup)
        with tc.tile_pool(name="pe1", bufs=4) as pe1:
            st1 = [None]

            def emit1(p, ps):
                i = p % F
                if i == 0:
                    st1[0] = pe1.tile((128, F // 2, 256), f16, name='st1')
                nc.scalar.activation(
                    st1[0][:, i // 2, (i % 2) * 128:(i % 2 + 1) * 128],
                    ps[:], copyf)
                if i == F - 1:
                    q, roff = pmeta(p - F + 1)
                    prow = roff // 2
                    nc.sync.dma_start(
                        P1[q].ap()[prow:prow + (F // 2) * 128, :]
                        .rearrange("(hi p) f -> p hi f", hi=F // 2), st1[0][:])
            def rs1(q):
                nc.gpsimd.collective_compute(
                    "ReduceScatter", mybir.AluOpType.add,
                    replica_groups=groups,
                    ins=[P1[q].ap()], outs=[H1[q].ap()])
            scatter_layer(t1, c.HID, emit1, qhook=rs1)

        with tc.tile_pool(name="ph1", bufs=3) as ph1, \
             tc.tile_pool(name="pds", bufs=3) as pds, \
             tc.tile_pool(name="pst", bufs=2, space="PSUM") as pst, \
             tc.tile_pool(name="psd", bufs=2, space="PSUM") as psd:
            for q in range(NQ):
                phase_b2_quarter(q, ph1, pds, pst, psd)

        # Layer-2 scatter -> P2 partials (quad/pair-packed rows)
        with tc.tile_pool(name="pe2", bufs=4) as pe2, \
             tc.tile_pool(name="po", bufs=3) as po:
            st2 = [None]

            def emit2(p, ps):
                q, roff = pmeta(p)
                pk = pk2[q]
                i = p % pk
                if i == 0:
                    st2[0] = pe2.tile((128, 256), f16, name='st2')
                nc.scalar.activation(
                    st2[0][:, i * c.OUT:(i + 1) * c.OUT], ps[:], copyf)
                if i == pk - 1:
                    grow = (roff // pk) - (pk - 1) * 128 // pk
                    nc.sync.dma_start(
                        P2[q].ap()[grow:grow + 128, :],
                        st2[0][:, 0:pk * c.OUT])
            def rs2(q):
                nc.gpsimd.collective_compute(
                    "ReduceScatter", mybir.AluOpType.add,
                    replica_groups=groups,
                    ins=[P2[q].ap()], outs=[O2[q].ap()])
                final_quarter(q, po)
            scatter_layer(t2, c.OUT, emit2, qhook=rs2)

    nc.compile()
    return nc


def make_inputs(cfg, features, edge_index, W1, W2):
    c = cfg
    colidx, rowloc, chunks, qoff, nch, cw, cnt128 = prep_edges(
        cfg, edge_index)
    iota2d = np.broadcast_to(np.arange(128, dtype=np.float16),
                             (128, 128)).copy()
    iotar = np.broadcast_to(np.arange(128, dtype=np.float16)[:, None],
                            (128, 8)).reshape(1, 1024)
    iotar = np.broadcast_to(iotar, (128, 1024)).copy()
    ident = np.eye(128, dtype=np.float16)
    w1 = np.ascontiguousarray(np.asarray(W1, np.float16))
    w2pad = np.zeros((c.HID, 128), np.float16)
    w2pad[:, :c.OUT] = np.asarray(W2, np.float16)
    in_maps = []
    for cc in range(c.NCORE):
        xc = np.asarray(features[cc * c.NPC:(cc + 1) * c.NPC], np.float32)
        xt = np.zeros((c.IN, c.NPP), np.float16)
        xt[:, :c.NPC] = xc.T.astype(np.float16)
        in_maps.append({
            "xT": np.ascontiguousarray(xt),
            "w1": w1, "w2p": w2pad, "iota": iota2d, "iotar": iotar,
            "ident": ident,
            "colidx": np.ascontiguousarray(colidx[cc]),
            "rowloc": np.ascontiguousarray(rowloc[cc]),
        })
    return in_maps, chunks, qoff, nch, cw, cnt128


_LAST_NC = None


def kernel(features, edge_index, W1, W2):
    global _LAST_NC
    cfg = CFG
    in_maps, chunks, qoff, nch, cw, cnt128 = make_inputs(
        cfg, features, edge_index, W1, W2)
    nc = build(cfg, chunks, qoff, nch, cw, cnt128)
    _LAST_NC = nc
    res = bass_utils.run_bass_kernel_spmd(
        nc, in_maps, core_ids=list(range(cfg.NCORE)))
    return np.concatenate(
        [res.results[cc]["out"] for cc in range(cfg.NCORE)], axis=0)



# revision 7
# speedup vs baseline: 1.0117x; 1.0002x over previous
"""2-layer GCN (gnn_message_passing) on 8 Trainium2 NeuronCores — v3.

Source-sharded: each core owns 12500 nodes (features + support rows local).
Per layer: support = X_c @ W (local, PE) -> local HBM table -> per GLOBAL dest
block: dma_gather local source rows (edges bucketed by dest block on host,
int16 local indices), scatter into the block via one-hot matmul in PSUM ->
partial-output tables (per dest quarter) -> chunked ReduceScatter(add) sums
the 8 cores' partials; each core receives its own 12500 rows. ReLU + W2
transform after RS1; layer-2 scatter reuses the same edge buffers (same edge
order) with width 64. Collectives are out-small (RS) and overlap the scatter
pipeline via per-quarter tensors.
"""
import sys
sys.path.insert(0, "/opt/trn_rl_repo")

import numpy as np
from contextlib import ExitStack

import concourse.bass as bass
import concourse.bacc as bacc
import concourse.tile as tile
from concourse import bass_utils
from concourse import mybir
from concourse.library_config import mlp

PADVAL = 200.0
GMAX = 8   # max 128-idx chunks per dma_gather call (HW limit: 1024 idx)
F = 4      # dest blocks per supergroup
NQ = 7     # ReduceScatter chunks (dest sevenths; 14 blocks each, even for pairing)


class Config:
    def __init__(self, n=100000, in_dim=256, hid=128, out_dim=64, ncore=8):
        self.N = n
        self.IN = in_dim
        self.HID = hid
        self.OUT = out_dim
        self.NCORE = ncore
        self.NPC = n // ncore
        assert self.NPC * ncore == n
        self.NB = (self.NPC + 127) // 128          # 98 local blocks
        self.NPP = self.NB * 128                   # 12544
        self.GB = ncore * self.NB                  # 784 global dest blocks
        self.KT = in_dim // 128
        # quarter sizes in local blocks: quad-packable regions + runt
        self.QB = [16] * 6 + [2]
        assert sum(self.QB) == self.NB and len(self.QB) == NQ
        self.QSTART = [sum(self.QB[:q]) for q in range(NQ)]
        assert self.NPP <= 32767  # int16 gather idx


CFG = Config()


def prep_edges(cfg, edge_index):
    """Bucket each core's SOURCE-owned edges by global dest block in the
    quarter-major P-table order. Returns per-core colidx (16-wrapped int16
    local source row), rowloc (fp16 dest-row-in-block), uniform chunk counts
    per P-block, and chunk offsets."""
    c = cfg
    row = np.asarray(edge_index[0], dtype=np.int64)   # dest
    col = np.asarray(edge_index[1], dtype=np.int64)   # src
    score = col // c.NPC                              # owner core (source)
    sloc = (col - score * c.NPC).astype(np.int16)     # gather row in t-table
    dcore = row // c.NPC
    dl = row - dcore * c.NPC
    db = dl // 128                                    # dest local block
    rib = (dl % 128).astype(np.float16)
    # quarter of dest block
    qid = np.searchsorted(np.asarray(c.QSTART + [c.NB]), db, side="right") - 1
    # P-block index: quarter-major, core-major inside
    qb = np.asarray(c.QB)[qid]
    blocks_before = np.asarray([sum(c.QB[:q]) for q in range(NQ)])[qid] * c.NCORE
    pblk = blocks_before + dcore * qb + (db - np.asarray(c.QSTART)[qid])

    key = score * c.GB + pblk
    order = np.argsort(key, kind="stable")
    key_s = key[order]
    sloc_s = sloc[order]
    rib_s = rib[order]
    counts = np.bincount(key_s, minlength=c.NCORE * c.GB).reshape(
        c.NCORE, c.GB)
    starts = np.concatenate([[0], np.cumsum(counts.reshape(-1))])

    chunks = -(-counts.max(axis=0) // 128)            # (GB,) uniform chunks
    nch = int(chunks.sum())
    cw = nch * 8
    qoff = np.concatenate([[0], np.cumsum(chunks)])   # chunk offset per block

    colidx = np.zeros((c.NCORE, 128, cw), np.int16)
    rowloc = np.full((c.NCORE, 128, nch), PADVAL, np.float16)
    for cc in range(c.NCORE):
        for p in range(c.GB):
            nk = int(chunks[p])
            if nk == 0:
                continue
            ki = cc * c.GB + p
            s, e = starts[ki], starts[ki + 1]
            cnt = e - s
            cap = nk * 128
            tl = np.zeros(cap, np.int16)
            tl[:cnt] = sloc_s[s:e]
            rb = np.full(cap, PADVAL, np.float16)
            rb[:cnt] = rib_s[s:e]
            qo = int(qoff[p])
            colidx[cc][:, qo * 8:(qo + nk) * 8] = np.tile(
                tl.reshape(-1, 16).T, (8, 1))
            rowloc[cc][:, qo:qo + nk] = rb.reshape(nk, 128).T
    cnt128 = counts.max(axis=0)                       # (GB,) true max counts
    return colidx, rowloc, chunks, qoff, nch, cw, cnt128


def build(cfg, chunks, qoff, nch, cw, cnt128):
    c = cfg
    nc = bacc.Bacc(None, target_bir_lowering=False, debug=False,
                   num_devices=c.NCORE, name="gcnv3", num_swdge_queues=4)
    f16, f32, i16 = mybir.dt.float16, mybir.dt.float32, mybir.dt.int16
    f8 = mybir.dt.float8e4
    relu = mybir.ActivationFunctionType.Relu
    copyf = mybir.ActivationFunctionType.Copy

    xT = nc.dram_tensor("xT", (c.IN, c.NPP), f16, kind="ExternalInput")
    w1 = nc.dram_tensor("w1", (c.IN, c.HID), f16, kind="ExternalInput")
    w2p = nc.dram_tensor("w2p", (c.HID, 128), f16, kind="ExternalInput")
    iota = nc.dram_tensor("iota", (128, 128), f16, kind="ExternalInput")
    iotar = nc.dram_tensor("iotar", (128, 128 * 8), f16, kind="ExternalInput")
    ident = nc.dram_tensor("ident", (128, 128), f16, kind="ExternalInput")
    colidx = nc.dram_tensor("colidx", (128, cw), i16, kind="ExternalInput")
    rowloc = nc.dram_tensor("rowloc", (128, nch), f16, kind="ExternalInput")
    out = nc.dram_tensor("out", (c.NPC, c.OUT), f32, kind="ExternalOutput")

    t1 = nc.dram_tensor("t1", (c.NPP, c.HID), f16, kind="Internal")
    t2 = nc.dram_tensor("t2", (c.NPP, 128), f16, kind="Internal")
    # per-quarter partial tables + RS outputs (separate tensors so each RS
    # only waits on its own quarter's writers)
    # L1 partials pack two dest blocks per row (512B rows -> full-rate DMA)
    P1 = [nc.dram_tensor(f"P1q{q}", (c.NCORE * (c.QB[q] // 2) * 128, 256), f16,
                         kind="Internal") for q in range(NQ)]
    H1 = [nc.dram_tensor(f"H1q{q}", ((c.QB[q] // 2) * 128, 256), f16,
                         kind="Internal") for q in range(NQ)]
    # L2 partials pack 4 (or 2, runt quarter) dest blocks per row
    pk2 = [4 if c.QB[q] % 4 == 0 else 2 for q in range(NQ)]
    P2 = [nc.dram_tensor(f"P2q{q}",
                         (c.NCORE * (c.QB[q] // pk2[q]) * 128,
                          c.OUT * pk2[q]), f16,
                         kind="Internal") for q in range(NQ)]
    O2 = [nc.dram_tensor(f"O2q{q}",
                         ((c.QB[q] // pk2[q]) * 128, c.OUT * pk2[q]), f16,
                         kind="Internal") for q in range(NQ)]

    groups = [list(range(c.NCORE))]
    iseq = mybir.AluOpType.is_equal

    # P-block -> (quarter, core, block-in-quarter) and row offset in P[q]
    def pmeta(p):
        acc = 0
        for q in range(NQ):
            nb = c.NCORE * c.QB[q]
            if p < acc + nb:
                j = p - acc
                return q, j * 128  # row offset within P[q]
            acc += nb
        raise AssertionError

    with ExitStack() as ctx:
        tc = ctx.enter_context(tile.TileContext(nc))
        nc.gpsimd.load_library(mlp)
        cpool = ctx.enter_context(tc.tile_pool(name="const", bufs=1))
        # shared gather pool, warmed once (hidden under phase A): trimmed
        # calls leave tail rows unwritten, which must be stale-finite
        pg = ctx.enter_context(tc.tile_pool(name="pg", bufs=24))
        for _ in range(24):
            wt = pg.tile((128, GMAX, 128), mybir.dt.float16, name="gt")
            nc.vector.memset(wt[:], 0.0)
        iota_sb = cpool.tile((128, 128), f16, tag="iota")
        nc.sync.dma_start(iota_sb[:], iota[:])
        iotar_sb = cpool.tile((128, 128, 8), f16, tag="iotar")
        nc.sync.dma_start(iotar_sb[:].rearrange("p r c -> p (r c)"), iotar[:])
        ident_sb = cpool.tile((128, 128), f16, tag="ident")
        nc.sync.dma_start(ident_sb[:], ident[:])
        w2_sb = cpool.tile((c.HID, 128), f16, tag="w2")
        nc.sync.dma_start(w2_sb[:], w2p[:])

        # Phase A: t1 = X_c @ W1 (local support)
        with tc.tile_pool(name="pa", bufs=1) as pa, \
             tc.tile_pool(name="pas", bufs=3) as pas, \
             tc.tile_pool(name="psa", bufs=2, space="PSUM") as psa:
            xk, w1k = [], []
            half = (c.NB // 2) * 128
            for k in range(c.KT):
                t = pa.tile((128, c.HID), f16, tag=f"w{k}", name=f"w1k{k}")
                nc.sync.dma_start(t[:], w1.ap()[k * 128:(k + 1) * 128, :])
                w1k.append(t)
            for k in range(c.KT):
                # column halves: block matmuls start at 50% of the x load
                ta = pa.tile((128, half), f16, tag=f"xa{k}", name=f"xka{k}")
                nc.sync.dma_start(ta[:],
                                  xT.ap()[k * 128:(k + 1) * 128, 0:half])
                tb = pa.tile((128, c.NPP - half), f16, tag=f"xb{k}",
                             name=f"xkb{k}")
                nc.sync.dma_start(tb[:],
                                  xT.ap()[k * 128:(k + 1) * 128, half:])
                xk.append((ta, tb))
            for b0 in range(0, c.NB, 2):
                gn = min(2, c.NB - b0)
                s1 = pas.tile((128, 2, c.HID), f16, name='s1')
                for i in range(gn):
                    b = b0 + i
                    ps = psa.tile((128, c.HID), f32, space="PSUM")
                    for k in range(c.KT):
                        xt_, off = ((xk[k][0], 0) if (b + 1) * 128 <= half
                                    else (xk[k][1], half))
                        nc.tensor.matmul(
                            ps[:],
                            xt_[:, b * 128 - off:(b + 1) * 128 - off],
                            w1k[k][:], start=(k == 0), stop=(k == c.KT - 1))
                    nc.scalar.activation(s1[:, i, :], ps[:], copyf)
                nc.sync.dma_start(
                    t1.ap()[b0 * 128:(b0 + gn) * 128, :]
                    .rearrange("(blk p) f -> p blk f", blk=gn),
                    s1[:, 0:gn, :])

        # edge metadata loads queued after phase-A inputs (same SP DMA FIFO)
        rowloc_sb = cpool.tile((128, nch), f16, tag="rowloc")
        nc.sync.dma_start(rowloc_sb[:], rowloc[:])
        colidx_sb = cpool.tile((128, cw), i16, tag="colidx")
        nc.sync.dma_start(colidx_sb[:], colidx[:])

        # chunk index -> trailing-pad trim when a gather call ends exactly at
        # this chunk boundary (a cell's last chunk): skip the cell's pad
        # slots.  num_idxs need not be a multiple of 128; untouched tail rows
        # of the (reused, uniform-shape) pool buffers hold stale-but-finite
        # data that the zero one-hot columns nullify.  The first 24 calls per
        # layer stay untrimmed so every pool buffer is fully written once.
        endtrim = {}
        for b in range(c.GB):
            if chunks[b] > 0:
                endtrim[int(qoff[b + 1])] = int(chunks[b] * 128 - cnt128[b])

        # P-block index at which each quarter region ends
        qends = {}
        acc = 0
        for q in range(NQ):
            acc += c.NCORE * c.QB[q]
            qends[acc] = q

        def scatter_layer(tab, width, emit, qhook=None):
            """Gather+scatter all P-blocks from local table `tab`;
            emit(p, psum_ap) per finished block; qhook(q) fires inline after
            the last supergroup of quarter q so collectives dispatch from the
            Pool sequencer mid-layer instead of queuing behind all gathers."""
            gq = [0]
            with tc.tile_pool(name="poh", bufs=3) as poh, \
                 tc.tile_pool(name="pso", bufs=2, space="PSUM") as pso:
                for g0 in range(0, c.GB, F):
                    bs = list(range(g0, min(g0 + F, c.GB)))
                    wg = int(chunks[bs].sum())
                    qg0 = int(qoff[bs[0]])
                    cblk = [b for b in bs for _ in range(int(chunks[b]))]
                    first, last = {}, {}
                    for ci, b in enumerate(cblk):
                        first.setdefault(b, ci)
                        last[b] = ci
                    psb = {b: pso.tile((128, width), f32, space="PSUM",
                                       tag=f"ps{b - g0}", name=f"ps{b - g0}")
                           for b in bs}
                    for b in bs:
                        if b not in first:
                            zt = pg.tile((128, width), f16)
                            nc.vector.memset(zt[:], 0.0)
                            nc.tensor.matmul(psb[b][:], ident_sb[:, 0:width],
                                             zt[:], start=True, stop=True)
                    if wg > 0:
                        oh = poh.tile((128, wg, 128), f16)
                        nc.vector.tensor_tensor(
                            out=oh[:],
                            in0=rowloc_sb[:, qg0:qg0 + wg].unsqueeze(2)
                                .to_broadcast((128, wg, 128)),
                            in1=iota_sb[:].unsqueeze(1)
                                .to_broadcast((128, wg, 128)),
                            op=iseq)
                        # choose call windows (<= GMAX chunks, minimal
                        # count) maximizing endings on cell boundaries so
                        # their trailing pads can be trimmed
                        K = -(-wg // GMAX)
                        dpk = {0: (0, [])}
                        ends = None
                        for _ in range(K):
                            nxt = {}
                            for pos, (t, es) in dpk.items():
                                for e in range(pos + 1,
                                               min(pos + GMAX, wg) + 1):
                                    tt = t + endtrim.get(qg0 + e, 0)
                                    if e not in nxt or nxt[e][0] < tt:
                                        nxt[e] = (tt, es + [e])
                            dpk = nxt
                            if wg in dpk:
                                ends = dpk[wg][1]
                                break
                        assert ends is not None
                        for wi, e in enumerate(ends):
                            s0 = ends[wi - 1] if wi else 0
                            sn = e - s0
                            trim = endtrim.get(qg0 + e, 0)
                            nidx = sn * 128 - trim
                            gt = pg.tile((128, GMAX, 128), f16, name="gt")
                            nc.gpsimd.dma_gather(
                                gt[:, 0:sn, :], tab.ap(),
                                colidx_sb[:, (qg0 + s0) * 8:
                                          (qg0 + s0 + sn) * 8],
                                nidx, nidx, 128,
                                queue_num=gq[0] % 4)
                            gq[0] += 1
                            for j in range(sn):
                                ci = s0 + j
                                b = cblk[ci]
                                nc.tensor.matmul(
                                    psb[b][:], oh[:, ci, :],
                                    gt[:, j, 0:width],
                                    start=(ci == first[b]),
                                    stop=(ci == last[b]))
                    for b in bs:
                        emit(b, psb[b])
                    if qhook is not None and (g0 + F) in qends:
                        qhook(qends[g0 + F])

        # Phase B2: per local block: relu(h1) -> transpose -> @W2 -> t2
        # (loads/stores batched 4 blocks per DMA; emitted per quarter from
        # the RS1 hook so the work hides inside the L1 scatter phase)
        def phase_b2_quarter(q, ph1, pds, pst, psd):
            if True:
                for lb0 in range(0, c.QB[q], 4):
                    gn = min(4, c.QB[q] - lb0)
                    nhi = gn // 2
                    hb = ph1.tile((128, 2, 256), f16, tag="hb", name="hb")
                    nc.sync.dma_start(
                        hb[:, 0:nhi, :],
                        H1[q].ap()[(lb0 // 2) * 128:(lb0 // 2 + nhi) * 128, :]
                        .rearrange("(hi p) f -> p hi f", hi=nhi))
                    s2 = pds.tile((128, 4, 128), f16, name='s2')
                    h1b4 = ph1.tile((128, 2, 256), f16, tag="h1b",
                                    name="h1b")
                    nc.scalar.activation(h1b4[:, 0:nhi, :],
                                         hb[:, 0:nhi, :], relu)
                    for i in range(gn):
                        tp = pst.tile((c.HID, 128), f16, space="PSUM")
                        nc.tensor.transpose(
                            out=tp[:],
                            in_=h1b4[:, i // 2,
                                     (i % 2) * 128:(i % 2 + 1) * 128],
                            identity=ident_sb[:])
                        h1t = ph1.tile((c.HID, 128), f16, tag="h1t",
                                       name="h1t")
                        nc.vector.tensor_copy(h1t[:], tp[:])
                        ps2 = psd.tile((128, 64), f32, space="PSUM")
                        nc.tensor.matmul(ps2[:], h1t[:], w2_sb[:, 0:64],
                                         start=True, stop=True)
                        if i % 2 == 0:
                            nc.scalar.activation(s2[:, i, 0:64], ps2[:],
                                                 copyf)
                        else:
                            nc.vector.tensor_copy(s2[:, i, 0:64], ps2[:])
                    b0 = c.QSTART[q] + lb0
                    nc.sync.dma_start(
                        t2.ap()[b0 * 128:(b0 + gn) * 128, :]
                        .rearrange("(blk p) f -> p blk f", blk=gn),
                        s2[:, 0:gn, :])

        # Final: O2 -> fp32 out per quarter (from the RS2 hook)
        def final_quarter(q, po):
            if True:
                pk = pk2[q]
                for lb0 in range(0, c.QB[q], pk):
                    gn = pk
                    ot = po.tile((128, 4, c.OUT), f16, tag="ot", name="ot")
                    nc.sync.dma_start(
                        ot[:, 0:gn, :].rearrange("p blk f -> p (blk f)"),
                        O2[q].ap()[(lb0 // pk) * 128:
                                   (lb0 // pk + 1) * 128, :])
                    of = po.tile((128, 4, c.OUT), f32, tag="of", name="of")
                    nc.scalar.activation(of[:, 0:gn, :], ot[:, 0:gn, :], copyf)
                    b0 = c.QSTART[q] + lb0
                    full = min(gn, max(0, (c.NPC // 128) - b0))
                    if full > 0:
                        nc.sync.dma_start(
                            out.ap()[b0 * 128:(b0 + full) * 128, :]
                            .rearrange("(blk p) f -> p blk f", blk=full),
                            of[:, 0:full, :])
                    for i in range(full, gn):
                        b = b0 + i
                        rows = min(128, c.NPC - b * 128)
                        if rows > 0:
                            nc.sync.dma_start(
                                out.ap()[b * 128:b * 128 + rows, :],
                                of[0:rows, i, :])

        # Layer-1 scatter -> P1 partials (pair-packed, batched per supergroup)
        with tc.tile_pool(name="pe1", bufs=4) as pe1:
            st1 = [None]

            def emit1(p, ps):
                i = p % F
                if i == 0:
                    st1[0] = pe1.tile((128, F // 2, 256), f16, name='st1')
                nc.scalar.activation(
                    st1[0][:, i // 2, (i % 2) * 128:(i % 2 + 1) * 128],
                    ps[:], copyf)
                if i == F - 1:
                    q, roff = pmeta(p - F + 1)
                    prow = roff // 2
                    nc.sync.dma_start(
                        P1[q].ap()[prow:prow + (F // 2) * 128, :]
                        .rearrange("(hi p) f -> p hi f", hi=F // 2), st1[0][:])
            def rs1(q):
                nc.gpsimd.collective_compute(
                    "ReduceScatter", mybir.AluOpType.add,
                    replica_groups=groups,
                    ins=[P1[q].ap()], outs=[H1[q].ap()])
            scatter_layer(t1, c.HID, emit1, qhook=rs1)

        with tc.tile_pool(name="ph1", bufs=3) as ph1, \
             tc.tile_pool(name="pds", bufs=3) as pds, \
             tc.tile_pool(name="pst", bufs=2, space="PSUM") as pst, \
             tc.tile_pool(name="psd", bufs=2, space="PSUM") as psd:
            for q in range(NQ):
                phase_b2_quarter(q, ph1, pds, pst, psd)

        # Layer-2 scatter -> P2 partials (quad/pair-packed rows)
        with tc.tile_pool(name="pe2", bufs=4) as pe2, \
             tc.tile_pool(name="po", bufs=3) as po:
            st2 = [None]

            def emit2(p, ps):
                q, roff = pmeta(p)
                pk = pk2[q]
                i = p % pk
                if i == 0:
                    st2[0] = pe2.tile((128, 256), f16, name='st2')
                nc.scalar.activation(
                    st2[0][:, i * c.OUT:(i + 1) * c.OUT], ps[:], copyf)
                if i == pk - 1:
                    grow = (roff // pk) - (pk - 1) * 128 // pk
                    nc.sync.dma_start(
                        P2[q].ap()[grow:grow + 128, :],
                        st2[0][:, 0:pk * c.OUT])
            def rs2(q):
                nc.gpsimd.collective_compute(
                    "ReduceScatter", mybir.AluOpType.add,
                    replica_groups=groups,
                    ins=[P2[q].ap()], outs=[O2[q].ap()])
                final_quarter(q, po)
            scatter_layer(t2, c.OUT, emit2, qhook=rs2)

    nc.compile()
    return nc


def make_inputs(cfg, features, edge_index, W1, W2):
    c = cfg
    colidx, rowloc, chunks, qoff, nch, cw, cnt128 = prep_edges(
        cfg, edge_index)
    iota2d = np.broadcast_to(np.arange(128, dtype=np.float16),
                             (128, 128)).copy()
    iotar = np.broadcast_to(np.arange(128, dtype=np.float16)[:, None],
                            (128, 8)).reshape(1, 1024)
    iotar = np.broadcast_to(iotar, (128, 1024)).copy()
    ident = np.eye(128, dtype=np.float16)
    w1 = np.ascontiguousarray(np.asarray(W1, np.float16))
    w2pad = np.zeros((c.HID, 128), np.float16)
    w2pad[:, :c.OUT] = np.asarray(W2, np.float16)
    in_maps = []
    for cc in range(c.NCORE):
        xc = np.asarray(features[cc * c.NPC:(cc + 1) * c.NPC], np.float32)
        xt = np.zeros((c.IN, c.NPP), np.float16)
        xt[:, :c.NPC] = xc.T.astype(np.float16)
        in_maps.append({
            "xT": np.ascontiguousarray(xt),
            "w1": w1, "w2p": w2pad, "iota": iota2d, "iotar": iotar,
            "ident": ident,
            "colidx": np.ascontiguousarray(colidx[cc]),
            "rowloc": np.ascontiguousarray(rowloc[cc]),
        })
    return in_maps, chunks, qoff, nch, cw, cnt128


_LAST_NC = None


def kernel(features, edge_index, W1, W2):
    global _LAST_NC
    cfg = CFG
    in_maps, chunks, qoff, nch, cw, cnt128 = make_inputs(
        cfg, features, edge_index, W1, W2)
    nc = build(cfg, chunks, qoff, nch, cw, cnt128)
    _LAST_NC = nc
    res = bass_utils.run_bass_kernel_spmd(
        nc, in_maps, core_ids=list(range(cfg.NCORE)))
    return np.concatenate(
        [res.results[cc]["out"] for cc in range(cfg.NCORE)], axis=0)



# revision 8
# speedup vs baseline: 1.0148x; 1.0031x over previous
"""2-layer GCN (gnn_message_passing) on 8 Trainium2 NeuronCores — v3.

Source-sharded: each core owns 12500 nodes (features + support rows local).
Per layer: support = X_c @ W (local, PE) -> local HBM table -> per GLOBAL dest
block: dma_gather local source rows (edges bucketed by dest block on host,
int16 local indices), scatter into the block via one-hot matmul in PSUM ->
partial-output tables (per dest quarter) -> chunked ReduceScatter(add) sums
the 8 cores' partials; each core receives its own 12500 rows. ReLU + W2
transform after RS1; layer-2 scatter reuses the same edge buffers (same edge
order) with width 64. Collectives are out-small (RS) and overlap the scatter
pipeline via per-quarter tensors.
"""
import sys
sys.path.insert(0, "/opt/trn_rl_repo")

import numpy as np
from contextlib import ExitStack

import concourse.bass as bass
import concourse.bacc as bacc
import concourse.tile as tile
from concourse import bass_utils
from concourse import mybir
from concourse.library_config import mlp

PADVAL = 200.0
GMAX = 8   # max 128-idx chunks per dma_gather call (HW limit: 1024 idx)
F = 4      # dest blocks per supergroup
NQ = 7     # ReduceScatter chunks (dest sevenths; 14 blocks each, even for pairing)


class Config:
    def __init__(self, n=100000, in_dim=256, hid=128, out_dim=64, ncore=8):
        self.N = n
        self.IN = in_dim
        self.HID = hid
        self.OUT = out_dim
        self.NCORE = ncore
        self.NPC = n // ncore
        assert self.NPC * ncore == n
        self.NB = (self.NPC + 127) // 128          # 98 local blocks
        self.NPP = self.NB * 128                   # 12544
        self.GB = ncore * self.NB                  # 784 global dest blocks
        self.KT = in_dim // 128
        # quarter sizes in local blocks: quad-packable regions + runt
        self.QB = [16] * 6 + [2]
        assert sum(self.QB) == self.NB and len(self.QB) == NQ
        self.QSTART = [sum(self.QB[:q]) for q in range(NQ)]
        assert self.NPP <= 32767  # int16 gather idx


CFG = Config()


def prep_edges(cfg, edge_index):
    """Bucket each core's SOURCE-owned edges by global dest block in the
    quarter-major P-table order. Returns per-core colidx (16-wrapped int16
    local source row), rowloc (fp16 dest-row-in-block), uniform chunk counts
    per P-block, and chunk offsets."""
    c = cfg
    row = np.asarray(edge_index[0], dtype=np.int64)   # dest
    col = np.asarray(edge_index[1], dtype=np.int64)   # src
    score = col // c.NPC                              # owner core (source)
    sloc = (col - score * c.NPC).astype(np.int16)     # gather row in t-table
    dcore = row // c.NPC
    dl = row - dcore * c.NPC
    db = dl // 128                                    # dest local block
    rib = (dl % 128).astype(np.float16)
    # quarter of dest block
    qid = np.searchsorted(np.asarray(c.QSTART + [c.NB]), db, side="right") - 1
    # P-block index: quarter-major, core-major inside
    qb = np.asarray(c.QB)[qid]
    blocks_before = np.asarray([sum(c.QB[:q]) for q in range(NQ)])[qid] * c.NCORE
    pblk = blocks_before + dcore * qb + (db - np.asarray(c.QSTART)[qid])

    key = score * c.GB + pblk
    order = np.argsort(key, kind="stable")
    key_s = key[order]
    sloc_s = sloc[order]
    rib_s = rib[order]
    counts = np.bincount(key_s, minlength=c.NCORE * c.GB).reshape(
        c.NCORE, c.GB)
    starts = np.concatenate([[0], np.cumsum(counts.reshape(-1))])

    chunks = -(-counts.max(axis=0) // 128)            # (GB,) uniform chunks
    nch = int(chunks.sum())
    cw = nch * 8
    qoff = np.concatenate([[0], np.cumsum(chunks)])   # chunk offset per block

    colidx = np.zeros((c.NCORE, 128, cw), np.int16)
    rowloc = np.full((c.NCORE, 128, nch), PADVAL, np.float16)
    for cc in range(c.NCORE):
        for p in range(c.GB):
            nk = int(chunks[p])
            if nk == 0:
                continue
            ki = cc * c.GB + p
            s, e = starts[ki], starts[ki + 1]
            cnt = e - s
            cap = nk * 128
            tl = np.zeros(cap, np.int16)
            tl[:cnt] = sloc_s[s:e]
            rb = np.full(cap, PADVAL, np.float16)
            rb[:cnt] = rib_s[s:e]
            qo = int(qoff[p])
            colidx[cc][:, qo * 8:(qo + nk) * 8] = np.tile(
                tl.reshape(-1, 16).T, (8, 1))
            rowloc[cc][:, qo:qo + nk] = rb.reshape(nk, 128).T
    cnt128 = counts.max(axis=0)                       # (GB,) true max counts
    return colidx, rowloc, chunks, qoff, nch, cw, cnt128


def build(cfg, chunks, qoff, nch, cw, cnt128):
    c = cfg
    nc = bacc.Bacc(None, target_bir_lowering=False, debug=False,
                   num_devices=c.NCORE, name="gcnv3", num_swdge_queues=4)
    f16, f32, i16 = mybir.dt.float16, mybir.dt.float32, mybir.dt.int16
    f8 = mybir.dt.float8e4
    relu = mybir.ActivationFunctionType.Relu
    copyf = mybir.ActivationFunctionType.Copy

    xT = nc.dram_tensor("xT", (c.IN, c.NPP), f16, kind="ExternalInput")
    w1 = nc.dram_tensor("w1", (c.IN, c.HID), f16, kind="ExternalInput")
    w2p = nc.dram_tensor("w2p", (c.HID, 128), f16, kind="ExternalInput")
    iota = nc.dram_tensor("iota", (128, 128), f16, kind="ExternalInput")
    iotar = nc.dram_tensor("iotar", (128, 128 * 8), f16, kind="ExternalInput")
    ident = nc.dram_tensor("ident", (128, 128), f16, kind="ExternalInput")
    colidx = nc.dram_tensor("colidx", (128, cw), i16, kind="ExternalInput")
    rowloc = nc.dram_tensor("rowloc", (128, nch), f16, kind="ExternalInput")
    out = nc.dram_tensor("out", (c.NPC, c.OUT), f32, kind="ExternalOutput")

    t1 = nc.dram_tensor("t1", (c.NPP, c.HID), f16, kind="Internal")
    t2 = nc.dram_tensor("t2", (c.NPP, 128), f16, kind="Internal")
    # per-quarter partial tables + RS outputs (separate tensors so each RS
    # only waits on its own quarter's writers)
    # L1 partials pack two dest blocks per row (512B rows -> full-rate DMA)
    P1 = [nc.dram_tensor(f"P1q{q}", (c.NCORE * (c.QB[q] // 2) * 128, 256), f16,
                         kind="Internal") for q in range(NQ)]
    H1 = [nc.dram_tensor(f"H1q{q}", ((c.QB[q] // 2) * 128, 256), f16,
                         kind="Internal") for q in range(NQ)]
    # L2 partials pack 4 (or 2, runt quarter) dest blocks per row
    pk2 = [4 if c.QB[q] % 4 == 0 else 2 for q in range(NQ)]
    P2 = [nc.dram_tensor(f"P2q{q}",
                         (c.NCORE * (c.QB[q] // pk2[q]) * 128,
                          c.OUT * pk2[q]), f16,
                         kind="Internal") for q in range(NQ)]
    O2 = [nc.dram_tensor(f"O2q{q}",
                         ((c.QB[q] // pk2[q]) * 128, c.OUT * pk2[q]), f16,
                         kind="Internal") for q in range(NQ)]

    groups = [list(range(c.NCORE))]
    iseq = mybir.AluOpType.is_equal

    # P-block -> (quarter, core, block-in-quarter) and row offset in P[q]
    def pmeta(p):
        acc = 0
        for q in range(NQ):
            nb = c.NCORE * c.QB[q]
            if p < acc + nb:
                j = p - acc
                return q, j * 128  # row offset within P[q]
            acc += nb
        raise AssertionError

    with ExitStack() as ctx:
        tc = ctx.enter_context(tile.TileContext(nc))
        nc.gpsimd.load_library(mlp)
        cpool = ctx.enter_context(tc.tile_pool(name="const", bufs=1))
        # shared gather pool, warmed once (hidden under phase A): trimmed
        # calls leave tail rows unwritten, which must be stale-finite
        pg = ctx.enter_context(tc.tile_pool(name="pg", bufs=24))
        for _ in range(24):
            wt = pg.tile((128, GMAX, 128), mybir.dt.float16, name="gt")
            nc.vector.memset(wt[:], 0.0)
        iota_sb = cpool.tile((128, 128), f16, tag="iota")
        nc.sync.dma_start(iota_sb[:], iota[:])
        iotar_sb = cpool.tile((128, 128, 8), f16, tag="iotar")
        nc.sync.dma_start(iotar_sb[:].rearrange("p r c -> p (r c)"), iotar[:])
        ident_sb = cpool.tile((128, 128), f16, tag="ident")
        nc.sync.dma_start(ident_sb[:], ident[:])
        w2_sb = cpool.tile((c.HID, 128), f16, tag="w2")
        nc.sync.dma_start(w2_sb[:], w2p[:])

        # Phase A: t1 = X_c @ W1 (local support)
        with tc.tile_pool(name="pa", bufs=1) as pa, \
             tc.tile_pool(name="pas", bufs=3) as pas, \
             tc.tile_pool(name="psa", bufs=2, space="PSUM") as psa:
            xk, w1k = [], []
            half = (c.NB // 2) * 128
            for k in range(c.KT):
                t = pa.tile((128, c.HID), f16, tag=f"w{k}", name=f"w1k{k}")
                nc.sync.dma_start(t[:], w1.ap()[k * 128:(k + 1) * 128, :])
                w1k.append(t)
            for k in range(c.KT):
                # column halves: block matmuls start at 50% of the x load
                ta = pa.tile((128, half), f16, tag=f"xa{k}", name=f"xka{k}")
                nc.sync.dma_start(ta[:],
                                  xT.ap()[k * 128:(k + 1) * 128, 0:half])
                tb = pa.tile((128, c.NPP - half), f16, tag=f"xb{k}",
                             name=f"xkb{k}")
                nc.sync.dma_start(tb[:],
                                  xT.ap()[k * 128:(k + 1) * 128, half:])
                xk.append((ta, tb))
            for b0 in range(0, c.NB, 2):
                gn = min(2, c.NB - b0)
                s1 = pas.tile((128, 2, c.HID), f16, name='s1')
                for i in range(gn):
                    b = b0 + i
                    ps = psa.tile((128, c.HID), f32, space="PSUM")
                    for k in range(c.KT):
                        xt_, off = ((xk[k][0], 0) if (b + 1) * 128 <= half
                                    else (xk[k][1], half))
                        nc.tensor.matmul(
                            ps[:],
                            xt_[:, b * 128 - off:(b + 1) * 128 - off],
                            w1k[k][:], start=(k == 0), stop=(k == c.KT - 1))
                    nc.scalar.activation(s1[:, i, :], ps[:], copyf)
                nc.sync.dma_start(
                    t1.ap()[b0 * 128:(b0 + gn) * 128, :]
                    .rearrange("(blk p) f -> p blk f", blk=gn),
                    s1[:, 0:gn, :])

        # edge metadata loads queued after phase-A inputs (same SP DMA FIFO)
        rowloc_sb = cpool.tile((128, nch), f16, tag="rowloc")
        nc.sync.dma_start(rowloc_sb[:], rowloc[:])
        colidx_sb = cpool.tile((128, cw), i16, tag="colidx")
        nc.sync.dma_start(colidx_sb[:], colidx[:])

        # chunk index -> trailing-pad trim when a gather call ends exactly at
        # this chunk boundary (a cell's last chunk): skip the cell's pad
        # slots.  num_idxs need not be a multiple of 128; untouched tail rows
        # of the (reused, uniform-shape) pool buffers hold stale-but-finite
        # data that the zero one-hot columns nullify.  The first 24 calls per
        # layer stay untrimmed so every pool buffer is fully written once.
        endtrim = {}
        for b in range(c.GB):
            if chunks[b] > 0:
                endtrim[int(qoff[b + 1])] = int(chunks[b] * 128 - cnt128[b])

        # P-block index at which each quarter region ends
        qends = {}
        acc = 0
        for q in range(NQ):
            acc += c.NCORE * c.QB[q]
            qends[acc] = q

        def scatter_layer(tab, width, emit, qhook=None):
            """Gather+scatter all P-blocks from local table `tab`;
            emit(p, psum_ap) per finished block; qhook(q) fires inline after
            the last supergroup of quarter q so collectives dispatch from the
            Pool sequencer mid-layer instead of queuing behind all gathers."""
            gq = [0]
            with tc.tile_pool(name="poh", bufs=3) as poh, \
                 tc.tile_pool(name="pso", bufs=2, space="PSUM") as pso:
                for g0 in range(0, c.GB, F):
                    bs = list(range(g0, min(g0 + F, c.GB)))
                    wg = int(chunks[bs].sum())
                    qg0 = int(qoff[bs[0]])
                    cblk = [b for b in bs for _ in range(int(chunks[b]))]
                    first, last = {}, {}
                    for ci, b in enumerate(cblk):
                        first.setdefault(b, ci)
                        last[b] = ci
                    psb = {b: pso.tile((128, width), f32, space="PSUM",
                                       tag=f"ps{b - g0}", name=f"ps{b - g0}")
                           for b in bs}
                    for b in bs:
                        if b not in first:
                            zt = pg.tile((128, width), f16)
                            nc.vector.memset(zt[:], 0.0)
                            nc.tensor.matmul(psb[b][:], ident_sb[:, 0:width],
                                             zt[:], start=True, stop=True)
                    if wg > 0:
                        oh = poh.tile((128, wg, 128), f16)
                        nc.vector.tensor_tensor(
                            out=oh[:],
                            in0=rowloc_sb[:, qg0:qg0 + wg].unsqueeze(2)
                                .to_broadcast((128, wg, 128)),
                            in1=iota_sb[:].unsqueeze(1)
                                .to_broadcast((128, wg, 128)),
                            op=iseq)
                        # choose call windows (<= GMAX chunks, minimal
                        # count) maximizing endings on cell boundaries so
                        # their trailing pads can be trimmed
                        K = -(-wg // GMAX)
                        dpk = {0: (0, [])}
                        ends = None
                        for _ in range(K):
                            nxt = {}
                            for pos, (t, es) in dpk.items():
                                for e in range(pos + 1,
                                               min(pos + GMAX, wg) + 1):
                                    tt = t + endtrim.get(qg0 + e, 0)
                                    if e not in nxt or nxt[e][0] < tt:
                                        nxt[e] = (tt, es + [e])
                            dpk = nxt
                            if wg in dpk:
                                ends = dpk[wg][1]
                                break
                        assert ends is not None
                        for wi, e in enumerate(ends):
                            s0 = ends[wi - 1] if wi else 0
                            sn = e - s0
                            trim = endtrim.get(qg0 + e, 0)
                            nidx = sn * 128 - trim
                            gt = pg.tile((128, GMAX, 128), f16, name="gt")
                            nc.gpsimd.dma_gather(
                                gt[:, 0:sn, :], tab.ap(),
                                colidx_sb[:, (qg0 + s0) * 8:
                                          (qg0 + s0 + sn) * 8],
                                nidx, nidx, 128,
                                queue_num=gq[0] % 4)
                            gq[0] += 1
                            for j in range(sn):
                                ci = s0 + j
                                b = cblk[ci]
                                nc.tensor.matmul(
                                    psb[b][:], oh[:, ci, :],
                                    gt[:, j, 0:width],
                                    start=(ci == first[b]),
                                    stop=(ci == last[b]))
                    for b in bs:
                        emit(b, psb[b])
                    if qhook is not None and (g0 + F) in qends:
                        qhook(qends[g0 + F])

        # Phase B2: per local block: relu(h1) -> transpose -> @W2 -> t2
        # (loads/stores batched 4 blocks per DMA; emitted per quarter from
        # the RS1 hook so the work hides inside the L1 scatter phase)
        def phase_b2_quarter(q, ph1, pds, pst, psd):
            if True:
                for lb0 in range(0, c.QB[q], 4):
                    gn = min(4, c.QB[q] - lb0)
                    nhi = gn // 2
                    hb = ph1.tile((128, 2, 256), f16, tag="hb", name="hb")
                    nc.sync.dma_start(
                        hb[:, 0:nhi, :],
                        H1[q].ap()[(lb0 // 2) * 128:(lb0 // 2 + nhi) * 128, :]
                        .rearrange("(hi p) f -> p hi f", hi=nhi))
                    s2 = pds.tile((128, 4, 128), f16, name='s2')
                    h1b4 = ph1.tile((128, 2, 256), f16, tag="h1b",
                                    name="h1b")
                    nc.scalar.activation(h1b4[:, 0:nhi, :],
                                         hb[:, 0:nhi, :], relu)
                    for i in range(gn):
                        tp = pst.tile((c.HID, 128), f16, space="PSUM")
                        nc.tensor.transpose(
                            out=tp[:],
                            in_=h1b4[:, i // 2,
                                     (i % 2) * 128:(i % 2 + 1) * 128],
                            identity=ident_sb[:])
                        h1t = ph1.tile((c.HID, 128), f16, tag="h1t",
                                       name="h1t")
                        nc.vector.tensor_copy(h1t[:], tp[:])
                        ps2 = psd.tile((128, 64), f32, space="PSUM")
                        nc.tensor.matmul(ps2[:], h1t[:], w2_sb[:, 0:64],
                                         start=True, stop=True)
                        if i % 2 == 0:
                            nc.scalar.activation(s2[:, i, 0:64], ps2[:],
                                                 copyf)
                        else:
                            nc.vector.tensor_copy(s2[:, i, 0:64], ps2[:])
                    b0 = c.QSTART[q] + lb0
                    nc.sync.dma_start(
                        t2.ap()[b0 * 128:(b0 + gn) * 128, :]
                        .rearrange("(blk p) f -> p blk f", blk=gn),
                        s2[:, 0:gn, :])

        # Final: O2 -> fp32 out per quarter (from the RS2 hook)
        def final_quarter(q, po):
            if True:
                pk = pk2[q]
                for lb0 in range(0, c.QB[q], pk):
                    gn = pk
                    ot = po.tile((128, 4, c.OUT), f16, tag="ot", name="ot")
                    nc.sync.dma_start(
                        ot[:, 0:gn, :].rearrange("p blk f -> p (blk f)"),
                        O2[q].ap()[(lb0 // pk) * 128:
                                   (lb0 // pk + 1) * 128, :])
                    of = po.tile((128, 4, c.OUT), f32, tag="of", name="of")
                    nc.vector.tensor_copy(of[:, 0:gn, :], ot[:, 0:gn, :])
                    b0 = c.QSTART[q] + lb0
                    full = min(gn, max(0, (c.NPC // 128) - b0))
                    if full > 0:
                        nc.sync.dma_start(
                            out.ap()[b0 * 128:(b0 + full) * 128, :]
                            .rearrange("(blk p) f -> p blk f", blk=full),
                            of[:, 0:full, :])
                    for i in range(full, gn):
                        b = b0 + i
                        rows = min(128, c.NPC - b * 128)
                        if rows > 0:
                            nc.sync.dma_start(
                                out.ap()[b * 128:b * 128 + rows, :],
                                of[0:rows, i, :])

        # Layer-1 scatter -> P1 partials (pair-packed, batched per supergroup)
        with tc.tile_pool(name="pe1", bufs=4) as pe1:
            st1 = [None]

            def emit1(p, ps):
                i = p % F
                if i == 0:
                    st1[0] = pe1.tile((128, F // 2, 256), f16, name='st1')
                nc.scalar.activation(
                    st1[0][:, i // 2, (i % 2) * 128:(i % 2 + 1) * 128],
                    ps[:], copyf)
                if i == F - 1:
                    q, roff = pmeta(p - F + 1)
                    prow = roff // 2
                    nc.sync.dma_start(
                        P1[q].ap()[prow:prow + (F // 2) * 128, :]
                        .rearrange("(hi p) f -> p hi f", hi=F // 2), st1[0][:])
            def rs1(q):
                nc.gpsimd.collective_compute(
                    "ReduceScatter", mybir.AluOpType.add,
                    replica_groups=groups,
                    ins=[P1[q].ap()], outs=[H1[q].ap()])
            scatter_layer(t1, c.HID, emit1, qhook=rs1)

        with tc.tile_pool(name="ph1", bufs=3) as ph1, \
             tc.tile_pool(name="pds", bufs=3) as pds, \
             tc.tile_pool(name="pst", bufs=2, space="PSUM") as pst, \
             tc.tile_pool(name="psd", bufs=2, space="PSUM") as psd:
            for q in range(NQ):
                phase_b2_quarter(q, ph1, pds, pst, psd)

        # Layer-2 scatter -> P2 partials (quad/pair-packed rows)
        with tc.tile_pool(name="pe2", bufs=4) as pe2, \
             tc.tile_pool(name="po", bufs=3) as po:
            st2 = [None]

            def emit2(p, ps):
                q, roff = pmeta(p)
                pk = pk2[q]
                i = p % pk
                if i == 0:
                    st2[0] = pe2.tile((128, 256), f16, name='st2')
                nc.scalar.activation(
                    st2[0][:, i * c.OUT:(i + 1) * c.OUT], ps[:], copyf)
                if i == pk - 1:
                    grow = (roff // pk) - (pk - 1) * 128 // pk
                    nc.sync.dma_start(
                        P2[q].ap()[grow:grow + 128, :],
                        st2[0][:, 0:pk * c.OUT])
            def rs2(q):
                nc.gpsimd.collective_compute(
                    "ReduceScatter", mybir.AluOpType.add,
                    replica_groups=groups,
                    ins=[P2[q].ap()], outs=[O2[q].ap()])
                final_quarter(q, po)
            scatter_layer(t2, c.OUT, emit2, qhook=rs2)

    nc.compile()
    return nc


def make_inputs(cfg, features, edge_index, W1, W2):
    c = cfg
    colidx, rowloc, chunks, qoff, nch, cw, cnt128 = prep_edges(
        cfg, edge_index)
    iota2d = np.broadcast_to(np.arange(128, dtype=np.float16),
                             (128, 128)).copy()
    iotar = np.broadcast_to(np.arange(128, dtype=np.float16)[:, None],
                            (128, 8)).reshape(1, 1024)
    iotar = np.broadcast_to(iotar, (128, 1024)).copy()
    ident = np.eye(128, dtype=np.float16)
    w1 = np.ascontiguousarray(np.asarray(W1, np.float16))
    w2pad = np.zeros((c.HID, 128), np.float16)
    w2pad[:, :c.OUT] = np.asarray(W2, np.float16)
    in_maps = []
    for cc in range(c.NCORE):
        xc = np.asarray(features[cc * c.NPC:(cc + 1) * c.NPC], np.float32)
        xt = np.zeros((c.IN, c.NPP), np.float16)
        xt[:, :c.NPC] = xc.T.astype(np.float16)
        in_maps.append({
            "xT": np.ascontiguousarray(xt),
            "w1": w1, "w2p": w2pad, "iota": iota2d, "iotar": iotar,
            "ident": ident,
            "colidx": np.ascontiguousarray(colidx[cc]),
            "rowloc": np.ascontiguousarray(rowloc[cc]),
        })
    return in_maps, chunks, qoff, nch, cw, cnt128


_LAST_NC = None


def kernel(features, edge_index, W1, W2):
    global _LAST_NC
    cfg = CFG
    in_maps, chunks, qoff, nch, cw, cnt128 = make_inputs(
        cfg, features, edge_index, W1, W2)
    nc = build(cfg, chunks, qoff, nch, cw, cnt128)
    _LAST_NC = nc
    res = bass_utils.run_bass_kernel_spmd(
        nc, in_maps, core_ids=list(range(cfg.NCORE)))
    return np.concatenate(
        [res.results[cc]["out"] for cc in range(cfg.NCORE)], axis=0)



# revision 9
# speedup vs baseline: 1.0265x; 1.0115x over previous
"""2-layer GCN (gnn_message_passing) on 8 Trainium2 NeuronCores — v3.

Source-sharded: each core owns 12500 nodes (features + support rows local).
Per layer: support = X_c @ W (local, PE) -> local HBM table -> per GLOBAL dest
block: dma_gather local source rows (edges bucketed by dest block on host,
int16 local indices), scatter into the block via one-hot matmul in PSUM ->
partial-output tables (per dest quarter) -> chunked ReduceScatter(add) sums
the 8 cores' partials; each core receives its own 12500 rows. ReLU + W2
transform after RS1; layer-2 scatter reuses the same edge buffers (same edge
order) with width 64. Collectives are out-small (RS) and overlap the scatter
pipeline via per-quarter tensors.
"""
import sys
sys.path.insert(0, "/opt/trn_rl_repo")

import numpy as np
from contextlib import ExitStack

import concourse.bass as bass
import concourse.bacc as bacc
import concourse.tile as tile
from concourse import bass_utils
from concourse import mybir
from concourse.library_config import mlp

PADVAL = 200.0
GMAX = 8   # max 128-idx chunks per dma_gather call (HW limit: 1024 idx)
F = 4      # dest blocks per supergroup
NQ = 7     # ReduceScatter chunks (dest sevenths; 14 blocks each, even for pairing)


class Config:
    def __init__(self, n=100000, in_dim=256, hid=128, out_dim=64, ncore=8):
        self.N = n
        self.IN = in_dim
        self.HID = hid
        self.OUT = out_dim
        self.NCORE = ncore
        self.NPC = n // ncore
        assert self.NPC * ncore == n
        self.NB = (self.NPC + 127) // 128          # 98 local blocks
        self.NPP = self.NB * 128                   # 12544
        self.GB = ncore * self.NB                  # 784 global dest blocks
        self.KT = in_dim // 128
        # quarter sizes in local blocks: quad-packable regions + runt
        self.QB = [16] * 6 + [2]
        assert sum(self.QB) == self.NB and len(self.QB) == NQ
        self.QSTART = [sum(self.QB[:q]) for q in range(NQ)]
        assert self.NPP <= 32767  # int16 gather idx


CFG = Config()


def prep_edges(cfg, edge_index):
    """Bucket each core's SOURCE-owned edges by global dest block in the
    quarter-major P-table order. Returns per-core colidx (16-wrapped int16
    local source row), rowloc (fp16 dest-row-in-block), uniform chunk counts
    per P-block, and chunk offsets."""
    c = cfg
    row = np.asarray(edge_index[0], dtype=np.int64)   # dest
    col = np.asarray(edge_index[1], dtype=np.int64)   # src
    score = col // c.NPC                              # owner core (source)
    sloc = (col - score * c.NPC).astype(np.int16)     # gather row in t-table
    dcore = row // c.NPC
    dl = row - dcore * c.NPC
    db = dl // 128                                    # dest local block
    rib = (dl % 128).astype(np.float16)
    # quarter of dest block
    qid = np.searchsorted(np.asarray(c.QSTART + [c.NB]), db, side="right") - 1
    # P-block index: quarter-major, core-major inside
    qb = np.asarray(c.QB)[qid]
    blocks_before = np.asarray([sum(c.QB[:q]) for q in range(NQ)])[qid] * c.NCORE
    pblk = blocks_before + dcore * qb + (db - np.asarray(c.QSTART)[qid])

    key = score * c.GB + pblk
    order = np.argsort(key, kind="stable")
    key_s = key[order]
    sloc_s = sloc[order]
    rib_s = rib[order]
    counts = np.bincount(key_s, minlength=c.NCORE * c.GB).reshape(
        c.NCORE, c.GB)
    starts = np.concatenate([[0], np.cumsum(counts.reshape(-1))])

    chunks = -(-counts.max(axis=0) // 128)            # (GB,) uniform chunks
    nch = int(chunks.sum())
    cw = nch * 8
    qoff = np.concatenate([[0], np.cumsum(chunks)])   # chunk offset per block

    colidx = np.zeros((c.NCORE, 128, cw), np.int16)
    rowloc = np.full((c.NCORE, 128, nch), PADVAL, np.float16)
    for cc in range(c.NCORE):
        for p in range(c.GB):
            nk = int(chunks[p])
            if nk == 0:
                continue
            ki = cc * c.GB + p
            s, e = starts[ki], starts[ki + 1]
            cnt = e - s
            cap = nk * 128
            tl = np.zeros(cap, np.int16)
            tl[:cnt] = sloc_s[s:e]
            rb = np.full(cap, PADVAL, np.float16)
            rb[:cnt] = rib_s[s:e]
            qo = int(qoff[p])
            colidx[cc][:, qo * 8:(qo + nk) * 8] = np.tile(
                tl.reshape(-1, 16).T, (8, 1))
            rowloc[cc][:, qo:qo + nk] = rb.reshape(nk, 128).T
    cnt128 = counts.max(axis=0)                       # (GB,) true max counts
    return colidx, rowloc, chunks, qoff, nch, cw, cnt128


def build(cfg, chunks, qoff, nch, cw, cnt128):
    c = cfg
    nc = bacc.Bacc(None, target_bir_lowering=False, debug=False,
                   num_devices=c.NCORE, name="gcnv3", num_swdge_queues=4)
    f16, f32, i16 = mybir.dt.float16, mybir.dt.float32, mybir.dt.int16
    f8 = mybir.dt.float8e4
    relu = mybir.ActivationFunctionType.Relu
    copyf = mybir.ActivationFunctionType.Copy

    xT = nc.dram_tensor("xT", (c.IN, c.NPP), f16, kind="ExternalInput")
    w1 = nc.dram_tensor("w1", (c.IN, c.HID), f16, kind="ExternalInput")
    w2p = nc.dram_tensor("w2p", (c.HID, 128), f16, kind="ExternalInput")
    iota = nc.dram_tensor("iota", (128, 128), f16, kind="ExternalInput")
    iotar = nc.dram_tensor("iotar", (128, 128 * 8), f16, kind="ExternalInput")
    ident = nc.dram_tensor("ident", (128, 128), f16, kind="ExternalInput")
    colidx = nc.dram_tensor("colidx", (128, cw), i16, kind="ExternalInput")
    rowloc = nc.dram_tensor("rowloc", (128, nch), f16, kind="ExternalInput")
    out = nc.dram_tensor("out", (c.NPC, c.OUT), f32, kind="ExternalOutput")

    t1 = nc.dram_tensor("t1", (c.NPP, c.HID), f16, kind="Internal")
    t2 = nc.dram_tensor("t2", (c.NPP, 128), f16, kind="Internal")
    # per-quarter partial tables + RS outputs (separate tensors so each RS
    # only waits on its own quarter's writers)
    # L1 partials pack two dest blocks per row (512B rows -> full-rate DMA)
    P1 = [nc.dram_tensor(f"P1q{q}", (c.NCORE * (c.QB[q] // 2) * 128, 256), f16,
                         kind="Internal") for q in range(NQ)]
    H1 = [nc.dram_tensor(f"H1q{q}", ((c.QB[q] // 2) * 128, 256), f16,
                         kind="Internal") for q in range(NQ)]
    # L2 partials pack 4 (or 2, runt quarter) dest blocks per row
    pk2 = [4 if c.QB[q] % 4 == 0 else 2 for q in range(NQ)]
    P2 = [nc.dram_tensor(f"P2q{q}",
                         (c.NCORE * (c.QB[q] // pk2[q]) * 128,
                          c.OUT * pk2[q]), f16,
                         kind="Internal") for q in range(NQ)]
    O2 = [nc.dram_tensor(f"O2q{q}",
                         ((c.QB[q] // pk2[q]) * 128, c.OUT * pk2[q]), f16,
                         kind="Internal") for q in range(NQ)]

    groups = [list(range(c.NCORE))]
    iseq = mybir.AluOpType.is_equal

    # P-block -> (quarter, core, block-in-quarter) and row offset in P[q]
    def pmeta(p):
        acc = 0
        for q in range(NQ):
            nb = c.NCORE * c.QB[q]
            if p < acc + nb:
                j = p - acc
                return q, j * 128  # row offset within P[q]
            acc += nb
        raise AssertionError

    with ExitStack() as ctx:
        tc = ctx.enter_context(tile.TileContext(nc))
        nc.gpsimd.load_library(mlp)
        cpool = ctx.enter_context(tc.tile_pool(name="const", bufs=1))
        # shared gather pool, warmed once (hidden under phase A): trimmed
        # calls leave tail rows unwritten, which must be stale-finite
        pg = ctx.enter_context(tc.tile_pool(name="pg", bufs=24))
        for _ in range(24):
            wt = pg.tile((128, GMAX, 128), mybir.dt.float16, name="gt")
            nc.vector.memset(wt[:], 0.0)
        iota_sb = cpool.tile((128, 128), f16, tag="iota")
        nc.sync.dma_start(iota_sb[:], iota[:])
        iotar_sb = cpool.tile((128, 128, 8), f16, tag="iotar")
        nc.sync.dma_start(iotar_sb[:].rearrange("p r c -> p (r c)"), iotar[:])
        ident_sb = cpool.tile((128, 128), f16, tag="ident")
        nc.sync.dma_start(ident_sb[:], ident[:])
        w2_sb = cpool.tile((c.HID, 128), f16, tag="w2")
        nc.sync.dma_start(w2_sb[:], w2p[:])

        # Phase A: t1 = X_c @ W1 (local support)
        with tc.tile_pool(name="pa", bufs=1) as pa, \
             tc.tile_pool(name="pas", bufs=3) as pas, \
             tc.tile_pool(name="psa", bufs=2, space="PSUM") as psa:
            xk, w1k = [], []
            half = (c.NB // 2) * 128
            for k in range(c.KT):
                t = pa.tile((128, c.HID), f16, tag=f"w{k}", name=f"w1k{k}")
                nc.sync.dma_start(t[:], w1.ap()[k * 128:(k + 1) * 128, :])
                w1k.append(t)
            for k in range(c.KT):
                # column halves: block matmuls start at 50% of the x load
                ta = pa.tile((128, half), f16, tag=f"xa{k}", name=f"xka{k}")
                nc.sync.dma_start(ta[:],
                                  xT.ap()[k * 128:(k + 1) * 128, 0:half])
                tb = pa.tile((128, c.NPP - half), f16, tag=f"xb{k}",
                             name=f"xkb{k}")
                nc.sync.dma_start(tb[:],
                                  xT.ap()[k * 128:(k + 1) * 128, half:])
                xk.append((ta, tb))
            for b0 in range(0, c.NB, 2):
                gn = min(2, c.NB - b0)
                s1 = pas.tile((128, 2, c.HID), f16, name='s1')
                for i in range(gn):
                    b = b0 + i
                    ps = psa.tile((128, c.HID), f32, space="PSUM")
                    for k in range(c.KT):
                        xt_, off = ((xk[k][0], 0) if (b + 1) * 128 <= half
                                    else (xk[k][1], half))
                        nc.tensor.matmul(
                            ps[:],
                            xt_[:, b * 128 - off:(b + 1) * 128 - off],
                            w1k[k][:], start=(k == 0), stop=(k == c.KT - 1))
                    nc.scalar.activation(s1[:, i, :], ps[:], copyf)
                nc.sync.dma_start(
                    t1.ap()[b0 * 128:(b0 + gn) * 128, :]
                    .rearrange("(blk p) f -> p blk f", blk=gn),
                    s1[:, 0:gn, :])

        # edge metadata loads queued after phase-A inputs (same SP DMA FIFO)
        rowloc_sb = cpool.tile((128, nch), f16, tag="rowloc")
        nc.sync.dma_start(rowloc_sb[:], rowloc[:])
        colidx_sb = cpool.tile((128, cw), i16, tag="colidx")
        nc.sync.dma_start(colidx_sb[:], colidx[:])

        # chunk index -> trailing-pad trim when a gather call ends exactly at
        # this chunk boundary (a cell's last chunk): skip the cell's pad
        # slots.  num_idxs need not be a multiple of 128; untouched tail rows
        # of the (reused, uniform-shape) pool buffers hold stale-but-finite
        # data that the zero one-hot columns nullify.  The first 24 calls per
        # layer stay untrimmed so every pool buffer is fully written once.
        endtrim = {}
        for b in range(c.GB):
            if chunks[b] > 0:
                endtrim[int(qoff[b + 1])] = int(chunks[b] * 128 - cnt128[b])

        # P-block index at which each quarter region ends
        qends = {}
        acc = 0
        for q in range(NQ):
            acc += c.NCORE * c.QB[q]
            qends[acc] = q

        def scatter_layer(tab, width, emit, qhook=None):
            """Gather+scatter all P-blocks from local table `tab`;
            emit(p, psum_ap) per finished block; qhook(q) fires inline after
            the last supergroup of quarter q so collectives dispatch from the
            Pool sequencer mid-layer instead of queuing behind all gathers."""
            gq = [0]
            with tc.tile_pool(name="poh", bufs=3) as poh, \
                 tc.tile_pool(name="pso", bufs=2, space="PSUM") as pso:
                for g0 in range(0, c.GB, F):
                    bs = list(range(g0, min(g0 + F, c.GB)))
                    wg = int(chunks[bs].sum())
                    qg0 = int(qoff[bs[0]])
                    cblk = [b for b in bs for _ in range(int(chunks[b]))]
                    first, last = {}, {}
                    for ci, b in enumerate(cblk):
                        first.setdefault(b, ci)
                        last[b] = ci
                    psb = {b: pso.tile((128, width), f32, space="PSUM",
                                       tag=f"ps{b - g0}", name=f"ps{b - g0}")
                           for b in bs}
                    for b in bs:
                        if b not in first:
                            zt = pg.tile((128, width), f16)
                            nc.vector.memset(zt[:], 0.0)
                            nc.tensor.matmul(psb[b][:], ident_sb[:, 0:width],
                                             zt[:], start=True, stop=True)
                    if wg > 0:
                        oh = poh.tile((128, wg, 128), f16)
                        nc.vector.tensor_tensor(
                            out=oh[:],
                            in0=rowloc_sb[:, qg0:qg0 + wg].unsqueeze(2)
                                .to_broadcast((128, wg, 128)),
                            in1=iota_sb[:].unsqueeze(1)
                                .to_broadcast((128, wg, 128)),
                            op=iseq)
                        # choose call windows (<= GMAX chunks, minimal
                        # count) maximizing endings on cell boundaries so
                        # their trailing pads can be trimmed
                        K = -(-wg // GMAX)
                        dpk = {0: (0, [])}
                        ends = None
                        for _ in range(K):
                            nxt = {}
                            for pos, (t, es) in dpk.items():
                                for e in range(pos + 1,
                                               min(pos + GMAX, wg) + 1):
                                    tt = t + endtrim.get(qg0 + e, 0)
                                    if e not in nxt or nxt[e][0] < tt:
                                        nxt[e] = (tt, es + [e])
                            dpk = nxt
                            if wg in dpk:
                                ends = dpk[wg][1]
                                break
                        assert ends is not None
                        for wi, e in enumerate(ends):
                            s0 = ends[wi - 1] if wi else 0
                            sn = e - s0
                            trim = endtrim.get(qg0 + e, 0)
                            nidx = sn * 128 - trim
                            gt = pg.tile((128, GMAX, 128), f16, name="gt")
                            nc.gpsimd.dma_gather(
                                gt[:, 0:sn, :], tab.ap(),
                                colidx_sb[:, (qg0 + s0) * 8:
                                          (qg0 + s0 + sn) * 8],
                                nidx, nidx, 128,
                                queue_num=gq[0] % 4)
                            gq[0] += 1
                            for j in range(sn):
                                ci = s0 + j
                                b = cblk[ci]
                                nc.tensor.matmul(
                                    psb[b][:], oh[:, ci, :],
                                    gt[:, j, 0:width],
                                    start=(ci == first[b]),
                                    stop=(ci == last[b]))
                    for b in bs:
                        emit(b, psb[b])
                    if qhook is not None and (g0 + F) in qends:
                        qhook(qends[g0 + F])

        # Phase B2: per local block: relu(h1) -> transpose -> @W2 -> t2
        # (loads/stores batched 4 blocks per DMA; emitted per quarter from
        # the RS1 hook so the work hides inside the L1 scatter phase)
        def phase_b2_quarter(q, ph1, pds, pst, psd):
            if True:
                for lb0 in range(0, c.QB[q], 4):
                    gn = min(4, c.QB[q] - lb0)
                    nhi = gn // 2
                    hb = ph1.tile((128, 2, 256), f16, tag="hb", name="hb")
                    nc.sync.dma_start(
                        hb[:, 0:nhi, :],
                        H1[q].ap()[(lb0 // 2) * 128:(lb0 // 2 + nhi) * 128, :]
                        .rearrange("(hi p) f -> p hi f", hi=nhi))
                    s2 = pds.tile((128, 4, 128), f16, name='s2')
                    h1b4 = ph1.tile((128, 2, 256), f16, tag="h1b",
                                    name="h1b")
                    nc.scalar.activation(h1b4[:, 0:nhi, :],
                                         hb[:, 0:nhi, :], relu)
                    for i in range(gn):
                        tp = pst.tile((c.HID, 128), f16, space="PSUM")
                        nc.tensor.transpose(
                            out=tp[:],
                            in_=h1b4[:, i // 2,
                                     (i % 2) * 128:(i % 2 + 1) * 128],
                            identity=ident_sb[:])
                        h1t = ph1.tile((c.HID, 128), f16, tag="h1t",
                                       name="h1t")
                        nc.vector.tensor_copy(h1t[:], tp[:])
                        ps2 = psd.tile((128, 64), f32, space="PSUM")
                        nc.tensor.matmul(ps2[:], h1t[:], w2_sb[:, 0:64],
                                         start=True, stop=True)
                        if i % 2 == 0:
                            nc.scalar.activation(s2[:, i, 0:64], ps2[:],
                                                 copyf)
                        else:
                            nc.vector.tensor_copy(s2[:, i, 0:64], ps2[:])
                    b0 = c.QSTART[q] + lb0
                    nc.sync.dma_start(
                        t2.ap()[b0 * 128:(b0 + gn) * 128, :]
                        .rearrange("(blk p) f -> p blk f", blk=gn),
                        s2[:, 0:gn, :])

        # Final: O2 -> fp32 out per quarter (from the RS2 hook)
        def final_quarter(q, po):
            if True:
                pk = pk2[q]
                for lb0 in range(0, c.QB[q], pk):
                    gn = pk
                    ot = po.tile((128, 4, c.OUT), f16, tag="ot", name="ot")
                    nc.sync.dma_start(
                        ot[:, 0:gn, :].rearrange("p blk f -> p (blk f)"),
                        O2[q].ap()[(lb0 // pk) * 128:
                                   (lb0 // pk + 1) * 128, :])
                    of = po.tile((128, 4, c.OUT), f32, tag="of", name="of")
                    nc.vector.tensor_copy(of[:, 0:gn, :], ot[:, 0:gn, :])
                    b0 = c.QSTART[q] + lb0
                    full = min(gn, max(0, (c.NPC // 128) - b0))
                    if full > 0:
                        nc.sync.dma_start(
                            out.ap()[b0 * 128:(b0 + full) * 128, :]
                            .rearrange("(blk p) f -> p blk f", blk=full),
                            of[:, 0:full, :])
                    for i in range(full, gn):
                        b = b0 + i
                        rows = min(128, c.NPC - b * 128)
                        if rows > 0:
                            nc.sync.dma_start(
                                out.ap()[b * 128:b * 128 + rows, :],
                                of[0:rows, i, :])

        # Layer-1 scatter -> P1 partials (pair-packed, batched per supergroup)
        with tc.tile_pool(name="pe1", bufs=4) as pe1:
            st1 = [None]

            def emit1(p, ps):
                i = p % F
                if i == 0:
                    st1[0] = pe1.tile((128, F // 2, 256), f16, name='st1')
                nc.scalar.activation(
                    st1[0][:, i // 2, (i % 2) * 128:(i % 2 + 1) * 128],
                    ps[:], copyf)
                if i == F - 1:
                    q, roff = pmeta(p - F + 1)
                    prow = roff // 2
                    nc.sync.dma_start(
                        P1[q].ap()[prow:prow + (F // 2) * 128, :]
                        .rearrange("(hi p) f -> p hi f", hi=F // 2), st1[0][:])
            def rs1(q):
                nc.gpsimd.collective_compute(
                    "ReduceScatter", mybir.AluOpType.add,
                    replica_groups=groups,
                    ins=[P1[q].ap()], outs=[H1[q].ap()])
            scatter_layer(t1, c.HID, emit1, qhook=rs1)

        with tc.tile_pool(name="ph1", bufs=4) as ph1, \
             tc.tile_pool(name="pds", bufs=4) as pds, \
             tc.tile_pool(name="pst", bufs=4, space="PSUM") as pst, \
             tc.tile_pool(name="psd", bufs=4, space="PSUM") as psd:
            for q in range(NQ):
                phase_b2_quarter(q, ph1, pds, pst, psd)

        # Layer-2 scatter -> P2 partials (quad/pair-packed rows)
        with tc.tile_pool(name="pe2", bufs=4) as pe2, \
             tc.tile_pool(name="po", bufs=3) as po:
            st2 = [None]

            def emit2(p, ps):
                q, roff = pmeta(p)
                pk = pk2[q]
                i = p % pk
                if i == 0:
                    st2[0] = pe2.tile((128, 256), f16, name='st2')
                nc.scalar.activation(
                    st2[0][:, i * c.OUT:(i + 1) * c.OUT], ps[:], copyf)
                if i == pk - 1:
                    grow = (roff // pk) - (pk - 1) * 128 // pk
                    nc.sync.dma_start(
                        P2[q].ap()[grow:grow + 128, :],
                        st2[0][:, 0:pk * c.OUT])
            def rs2(q):
                nc.gpsimd.collective_compute(
                    "ReduceScatter", mybir.AluOpType.add,
                    replica_groups=groups,
                    ins=[P2[q].ap()], outs=[O2[q].ap()])
                final_quarter(q, po)
            scatter_layer(t2, c.OUT, emit2, qhook=rs2)

    nc.compile()
    return nc


def make_inputs(cfg, features, edge_index, W1, W2):
    c = cfg
    colidx, rowloc, chunks, qoff, nch, cw, cnt128 = prep_edges(
        cfg, edge_index)
    iota2d = np.broadcast_to(np.arange(128, dtype=np.float16),
                             (128, 128)).copy()
    iotar = np.broadcast_to(np.arange(128, dtype=np.float16)[:, None],
                            (128, 8)).reshape(1, 1024)
    iotar = np.broadcast_to(iotar, (128, 1024)).copy()
    ident = np.eye(128, dtype=np.float16)
    w1 = np.ascontiguousarray(np.asarray(W1, np.float16))
    w2pad = np.zeros((c.HID, 128), np.float16)
    w2pad[:, :c.OUT] = np.asarray(W2, np.float16)
    in_maps = []
    for cc in range(c.NCORE):
        xc = np.asarray(features[cc * c.NPC:(cc + 1) * c.NPC], np.float32)
        xt = np.zeros((c.IN, c.NPP), np.float16)
        xt[:, :c.NPC] = xc.T.astype(np.float16)
        in_maps.append({
            "xT": np.ascontiguousarray(xt),
            "w1": w1, "w2p": w2pad, "iota": iota2d, "iotar": iotar,
            "ident": ident,
            "colidx": np.ascontiguousarray(colidx[cc]),
            "rowloc": np.ascontiguousarray(rowloc[cc]),
        })
    return in_maps, chunks, qoff, nch, cw, cnt128


_LAST_NC = None


def kernel(features, edge_index, W1, W2):
    global _LAST_NC
    cfg = CFG
    in_maps, chunks, qoff, nch, cw, cnt128 = make_inputs(
        cfg, features, edge_index, W1, W2)
    nc = build(cfg, chunks, qoff, nch, cw, cnt128)
    _LAST_NC = nc
    res = bass_utils.run_bass_kernel_spmd(
        nc, in_maps, core_ids=list(range(cfg.NCORE)))
    return np.concatenate(
        [res.results[cc]["out"] for cc in range(cfg.NCORE)], axis=0)



# revision 10
# speedup vs baseline: 1.0268x; 1.0003x over previous
"""2-layer GCN (gnn_message_passing) on 8 Trainium2 NeuronCores — v3.

Source-sharded: each core owns 12500 nodes (features + support rows local).
Per layer: support = X_c @ W (local, PE) -> local HBM table -> per GLOBAL dest
block: dma_gather local source rows (edges bucketed by dest block on host,
int16 local indices), scatter into the block via one-hot matmul in PSUM ->
partial-output tables (per dest quarter) -> chunked ReduceScatter(add) sums
the 8 cores' partials; each core receives its own 12500 rows. ReLU + W2
transform after RS1; layer-2 scatter reuses the same edge buffers (same edge
order) with width 64. Collectives are out-small (RS) and overlap the scatter
pipeline via per-quarter tensors.
"""
import sys
sys.path.insert(0, "/opt/trn_rl_repo")

import numpy as np
from contextlib import ExitStack

import concourse.bass as bass
import concourse.bacc as bacc
import concourse.tile as tile
from concourse import bass_utils
from concourse import mybir
from concourse.library_config import mlp

PADVAL = 200.0
GMAX = 8   # max 128-idx chunks per dma_gather call (HW limit: 1024 idx)
F = 4      # dest blocks per supergroup
NQ = 7     # ReduceScatter chunks (dest sevenths; 14 blocks each, even for pairing)


class Config:
    def __init__(self, n=100000, in_dim=256, hid=128, out_dim=64, ncore=8):
        self.N = n
        self.IN = in_dim
        self.HID = hid
        self.OUT = out_dim
        self.NCORE = ncore
        self.NPC = n // ncore
        assert self.NPC * ncore == n
        self.NB = (self.NPC + 127) // 128          # 98 local blocks
        self.NPP = self.NB * 128                   # 12544
        self.GB = ncore * self.NB                  # 784 global dest blocks
        self.KT = in_dim // 128
        # quarter sizes in local blocks: quad-packable regions + runt
        self.QB = [16] * 6 + [2]
        assert sum(self.QB) == self.NB and len(self.QB) == NQ
        self.QSTART = [sum(self.QB[:q]) for q in range(NQ)]
        assert self.NPP <= 32767  # int16 gather idx


CFG = Config()


def prep_edges(cfg, edge_index):
    """Bucket each core's SOURCE-owned edges by global dest block in the
    quarter-major P-table order. Returns per-core colidx (16-wrapped int16
    local source row), rowloc (fp16 dest-row-in-block), uniform chunk counts
    per P-block, and chunk offsets."""
    c = cfg
    row = np.asarray(edge_index[0], dtype=np.int64)   # dest
    col = np.asarray(edge_index[1], dtype=np.int64)   # src
    score = col // c.NPC                              # owner core (source)
    sloc = (col - score * c.NPC).astype(np.int16)     # gather row in t-table
    dcore = row // c.NPC
    dl = row - dcore * c.NPC
    db = dl // 128                                    # dest local block
    rib = (dl % 128).astype(np.float16)
    # quarter of dest block
    qid = np.searchsorted(np.asarray(c.QSTART + [c.NB]), db, side="right") - 1
    # P-block index: quarter-major, core-major inside
    qb = np.asarray(c.QB)[qid]
    blocks_before = np.asarray([sum(c.QB[:q]) for q in range(NQ)])[qid] * c.NCORE
    pblk = blocks_before + dcore * qb + (db - np.asarray(c.QSTART)[qid])

    key = score * c.GB + pblk
    order = np.argsort(key, kind="stable")
    key_s = key[order]
    sloc_s = sloc[order]
    rib_s = rib[order]
    counts = np.bincount(key_s, minlength=c.NCORE * c.GB).reshape(
        c.NCORE, c.GB)
    starts = np.concatenate([[0], np.cumsum(counts.reshape(-1))])

    chunks = -(-counts.max(axis=0) // 128)            # (GB,) uniform chunks
    nch = int(chunks.sum())
    cw = nch * 8
    qoff = np.concatenate([[0], np.cumsum(chunks)])   # chunk offset per block

    colidx = np.zeros((c.NCORE, 128, cw), np.int16)
    rowloc = np.full((c.NCORE, 128, nch), PADVAL, np.float16)
    for cc in range(c.NCORE):
        for p in range(c.GB):
            nk = int(chunks[p])
            if nk == 0:
                continue
            ki = cc * c.GB + p
            s, e = starts[ki], starts[ki + 1]
            cnt = e - s
            cap = nk * 128
            tl = np.zeros(cap, np.int16)
            tl[:cnt] = sloc_s[s:e]
            rb = np.full(cap, PADVAL, np.float16)
            rb[:cnt] = rib_s[s:e]
            qo = int(qoff[p])
            colidx[cc][:, qo * 8:(qo + nk) * 8] = np.tile(
                tl.reshape(-1, 16).T, (8, 1))
            rowloc[cc][:, qo:qo + nk] = rb.reshape(nk, 128).T
    cnt128 = counts.max(axis=0)                       # (GB,) true max counts
    return colidx, rowloc, chunks, qoff, nch, cw, cnt128


def build(cfg, chunks, qoff, nch, cw, cnt128):
    c = cfg
    nc = bacc.Bacc(None, target_bir_lowering=False, debug=False,
                   num_devices=c.NCORE, name="gcnv3", num_swdge_queues=4)
    f16, f32, i16 = mybir.dt.float16, mybir.dt.float32, mybir.dt.int16
    f8 = mybir.dt.float8e4
    relu = mybir.ActivationFunctionType.Relu
    copyf = mybir.ActivationFunctionType.Copy

    xT = nc.dram_tensor("xT", (c.IN, c.NPP), f16, kind="ExternalInput")
    w1 = nc.dram_tensor("w1", (c.IN, c.HID), f16, kind="ExternalInput")
    w2p = nc.dram_tensor("w2p", (c.HID, 128), f16, kind="ExternalInput")
    iota = nc.dram_tensor("iota", (128, 128), f16, kind="ExternalInput")
    iotar = nc.dram_tensor("iotar", (128, 128 * 8), f16, kind="ExternalInput")
    ident = nc.dram_tensor("ident", (128, 128), f16, kind="ExternalInput")
    colidx = nc.dram_tensor("colidx", (128, cw), i16, kind="ExternalInput")
    rowloc = nc.dram_tensor("rowloc", (128, nch), f16, kind="ExternalInput")
    out = nc.dram_tensor("out", (c.NPC, c.OUT), f32, kind="ExternalOutput")

    t1 = nc.dram_tensor("t1", (c.NPP, c.HID), f16, kind="Internal")
    t2 = nc.dram_tensor("t2", (c.NPP, 128), f16, kind="Internal")
    # per-quarter partial tables + RS outputs (separate tensors so each RS
    # only waits on its own quarter's writers)
    # L1 partials pack two dest blocks per row (512B rows -> full-rate DMA)
    P1 = [nc.dram_tensor(f"P1q{q}", (c.NCORE * (c.QB[q] // 2) * 128, 256), f16,
                         kind="Internal") for q in range(NQ)]
    H1 = [nc.dram_tensor(f"H1q{q}", ((c.QB[q] // 2) * 128, 256), f16,
                         kind="Internal") for q in range(NQ)]
    # L2 partials pack 4 (or 2, runt quarter) dest blocks per row
    pk2 = [4 if c.QB[q] % 4 == 0 else 2 for q in range(NQ)]
    P2 = [nc.dram_tensor(f"P2q{q}",
                         (c.NCORE * (c.QB[q] // pk2[q]) * 128,
                          c.OUT * pk2[q]), f16,
                         kind="Internal") for q in range(NQ)]
    O2 = [nc.dram_tensor(f"O2q{q}",
                         ((c.QB[q] // pk2[q]) * 128, c.OUT * pk2[q]), f16,
                         kind="Internal") for q in range(NQ)]

    groups = [list(range(c.NCORE))]
    iseq = mybir.AluOpType.is_equal

    # P-block -> (quarter, core, block-in-quarter) and row offset in P[q]
    def pmeta(p):
        acc = 0
        for q in range(NQ):
            nb = c.NCORE * c.QB[q]
            if p < acc + nb:
                j = p - acc
                return q, j * 128  # row offset within P[q]
            acc += nb
        raise AssertionError

    with ExitStack() as ctx:
        tc = ctx.enter_context(tile.TileContext(nc))
        nc.gpsimd.load_library(mlp)
        cpool = ctx.enter_context(tc.tile_pool(name="const", bufs=1))
        # shared gather pool, warmed once (hidden under phase A): trimmed
        # calls leave tail rows unwritten, which must be stale-finite
        pg = ctx.enter_context(tc.tile_pool(name="pg", bufs=24))
        for _ in range(24):
            wt = pg.tile((128, GMAX, 128), mybir.dt.float16, name="gt")
            nc.gpsimd.memset(wt[:], 0.0)
        iota_sb = cpool.tile((128, 128), f16, tag="iota")
        nc.sync.dma_start(iota_sb[:], iota[:])
        iotar_sb = cpool.tile((128, 128, 8), f16, tag="iotar")
        nc.sync.dma_start(iotar_sb[:].rearrange("p r c -> p (r c)"), iotar[:])
        ident_sb = cpool.tile((128, 128), f16, tag="ident")
        nc.sync.dma_start(ident_sb[:], ident[:])
        w2_sb = cpool.tile((c.HID, 128), f16, tag="w2")
        nc.sync.dma_start(w2_sb[:], w2p[:])

        # Phase A: t1 = X_c @ W1 (local support)
        with tc.tile_pool(name="pa", bufs=1) as pa, \
             tc.tile_pool(name="pas", bufs=5) as pas, \
             tc.tile_pool(name="psa", bufs=4, space="PSUM") as psa:
            xk, w1k = [], []
            half = (c.NB // 2) * 128
            for k in range(c.KT):
                t = pa.tile((128, c.HID), f16, tag=f"w{k}", name=f"w1k{k}")
                nc.sync.dma_start(t[:], w1.ap()[k * 128:(k + 1) * 128, :])
                w1k.append(t)
            for k in range(c.KT):
                # column halves: block matmuls start at 50% of the x load
                ta = pa.tile((128, half), f16, tag=f"xa{k}", name=f"xka{k}")
                nc.sync.dma_start(ta[:],
                                  xT.ap()[k * 128:(k + 1) * 128, 0:half])
                tb = pa.tile((128, c.NPP - half), f16, tag=f"xb{k}",
                             name=f"xkb{k}")
                nc.sync.dma_start(tb[:],
                                  xT.ap()[k * 128:(k + 1) * 128, half:])
                xk.append((ta, tb))
            for b0 in range(0, c.NB, 2):
                gn = min(2, c.NB - b0)
                s1 = pas.tile((128, 2, c.HID), f16, name='s1')
                for i in range(gn):
                    b = b0 + i
                    ps = psa.tile((128, c.HID), f32, space="PSUM")
                    for k in range(c.KT):
                        xt_, off = ((xk[k][0], 0) if (b + 1) * 128 <= half
                                    else (xk[k][1], half))
                        nc.tensor.matmul(
                            ps[:],
                            xt_[:, b * 128 - off:(b + 1) * 128 - off],
                            w1k[k][:], start=(k == 0), stop=(k == c.KT - 1))
                    if (b0 + i) % 2 == 0:
                        nc.scalar.activation(s1[:, i, :], ps[:], copyf)
                    else:
                        nc.vector.tensor_copy(s1[:, i, :], ps[:])
                nc.sync.dma_start(
                    t1.ap()[b0 * 128:(b0 + gn) * 128, :]
                    .rearrange("(blk p) f -> p blk f", blk=gn),
                    s1[:, 0:gn, :])

        # edge metadata loads queued after phase-A inputs (same SP DMA FIFO)
        rowloc_sb = cpool.tile((128, nch), f16, tag="rowloc")
        nc.sync.dma_start(rowloc_sb[:], rowloc[:])
        colidx_sb = cpool.tile((128, cw), i16, tag="colidx")
        nc.sync.dma_start(colidx_sb[:], colidx[:])

        # chunk index -> trailing-pad trim when a gather call ends exactly at
        # this chunk boundary (a cell's last chunk): skip the cell's pad
        # slots.  num_idxs need not be a multiple of 128; untouched tail rows
        # of the (reused, uniform-shape) pool buffers hold stale-but-finite
        # data that the zero one-hot columns nullify.  The first 24 calls per
        # layer stay untrimmed so every pool buffer is fully written once.
        endtrim = {}
        for b in range(c.GB):
            if chunks[b] > 0:
                endtrim[int(qoff[b + 1])] = int(chunks[b] * 128 - cnt128[b])

        # P-block index at which each quarter region ends
        qends = {}
        acc = 0
        for q in range(NQ):
            acc += c.NCORE * c.QB[q]
            qends[acc] = q

        def scatter_layer(tab, width, emit, qhook=None):
            """Gather+scatter all P-blocks from local table `tab`;
            emit(p, psum_ap) per finished block; qhook(q) fires inline after
            the last supergroup of quarter q so collectives dispatch from the
            Pool sequencer mid-layer instead of queuing behind all gathers."""
            gq = [0]
            with tc.tile_pool(name="poh", bufs=3) as poh, \
                 tc.tile_pool(name="pso", bufs=2, space="PSUM") as pso:
                for g0 in range(0, c.GB, F):
                    bs = list(range(g0, min(g0 + F, c.GB)))
                    wg = int(chunks[bs].sum())
                    qg0 = int(qoff[bs[0]])
                    cblk = [b for b in bs for _ in range(int(chunks[b]))]
                    first, last = {}, {}
                    for ci, b in enumerate(cblk):
                        first.setdefault(b, ci)
                        last[b] = ci
                    psb = {b: pso.tile((128, width), f32, space="PSUM",
                                       tag=f"ps{b - g0}", name=f"ps{b - g0}")
                           for b in bs}
                    for b in bs:
                        if b not in first:
                            zt = pg.tile((128, width), f16)
                            nc.vector.memset(zt[:], 0.0)
                            nc.tensor.matmul(psb[b][:], ident_sb[:, 0:width],
                                             zt[:], start=True, stop=True)
                    if wg > 0:
                        oh = poh.tile((128, wg, 128), f16)
                        nc.vector.tensor_tensor(
                            out=oh[:],
                            in0=rowloc_sb[:, qg0:qg0 + wg].unsqueeze(2)
                                .to_broadcast((128, wg, 128)),
                            in1=iota_sb[:].unsqueeze(1)
                                .to_broadcast((128, wg, 128)),
                            op=iseq)
                        # choose call windows (<= GMAX chunks, minimal
                        # count) maximizing endings on cell boundaries so
                        # their trailing pads can be trimmed
                        K = -(-wg // GMAX)
                        dpk = {0: (0, [])}
                        ends = None
                        for _ in range(K):
                            nxt = {}
                            for pos, (t, es) in dpk.items():
                                for e in range(pos + 1,
                                               min(pos + GMAX, wg) + 1):
                                    tt = t + endtrim.get(qg0 + e, 0)
                                    if e not in nxt or nxt[e][0] < tt:
                                        nxt[e] = (tt, es + [e])
                            dpk = nxt
                            if wg in dpk:
                                ends = dpk[wg][1]
                                break
                        assert ends is not None
                        for wi, e in enumerate(ends):
                            s0 = ends[wi - 1] if wi else 0
                            sn = e - s0
                            trim = endtrim.get(qg0 + e, 0)
                            nidx = sn * 128 - trim
                            gt = pg.tile((128, GMAX, 128), f16, name="gt")
                            nc.gpsimd.dma_gather(
                                gt[:, 0:sn, :], tab.ap(),
                                colidx_sb[:, (qg0 + s0) * 8:
                                          (qg0 + s0 + sn) * 8],
                                nidx, nidx, 128,
                                queue_num=gq[0] % 4)
                            gq[0] += 1
                            for j in range(sn):
                                ci = s0 + j
                                b = cblk[ci]
                                nc.tensor.matmul(
                                    psb[b][:], oh[:, ci, :],
                                    gt[:, j, 0:width],
                                    start=(ci == first[b]),
                                    stop=(ci == last[b]))
                    for b in bs:
                        emit(b, psb[b])
                    if qhook is not None and (g0 + F) in qends:
                        qhook(qends[g0 + F])

        # Phase B2: per local block: relu(h1) -> transpose -> @W2 -> t2
        # (loads/stores batched 4 blocks per DMA; emitted per quarter from
        # the RS1 hook so the work hides inside the L1 scatter phase)
        def phase_b2_quarter(q, ph1, pds, pst, psd):
            if True:
                for lb0 in range(0, c.QB[q], 4):
                    gn = min(4, c.QB[q] - lb0)
                    nhi = gn // 2
                    hb = ph1.tile((128, 2, 256), f16, tag="hb", name="hb")
                    nc.sync.dma_start(
                        hb[:, 0:nhi, :],
                        H1[q].ap()[(lb0 // 2) * 128:(lb0 // 2 + nhi) * 128, :]
                        .rearrange("(hi p) f -> p hi f", hi=nhi))
                    s2 = pds.tile((128, 4, 128), f16, name='s2')
                    h1b4 = ph1.tile((128, 2, 256), f16, tag="h1b",
                                    name="h1b")
                    nc.scalar.activation(h1b4[:, 0:nhi, :],
                                         hb[:, 0:nhi, :], relu)
                    for i in range(gn):
                        tp = pst.tile((c.HID, 128), f16, space="PSUM")
                        nc.tensor.transpose(
                            out=tp[:],
                            in_=h1b4[:, i // 2,
                                     (i % 2) * 128:(i % 2 + 1) * 128],
                            identity=ident_sb[:])
                        h1t = ph1.tile((c.HID, 128), f16, tag="h1t",
                                       name="h1t")
                        nc.vector.tensor_copy(h1t[:], tp[:])
                        ps2 = psd.tile((128, 64), f32, space="PSUM")
                        nc.tensor.matmul(ps2[:], h1t[:], w2_sb[:, 0:64],
                                         start=True, stop=True)
                        if i % 2 == 0:
                            nc.scalar.activation(s2[:, i, 0:64], ps2[:],
                                                 copyf)
                        else:
                            nc.vector.tensor_copy(s2[:, i, 0:64], ps2[:])
                    b0 = c.QSTART[q] + lb0
                    nc.sync.dma_start(
                        t2.ap()[b0 * 128:(b0 + gn) * 128, :]
                        .rearrange("(blk p) f -> p blk f", blk=gn),
                        s2[:, 0:gn, :])

        # Final: O2 -> fp32 out per quarter (from the RS2 hook)
        def final_quarter(q, po):
            if True:
                pk = pk2[q]
                for lb0 in range(0, c.QB[q], pk):
                    gn = pk
                    ot = po.tile((128, 4, c.OUT), f16, tag="ot", name="ot")
                    nc.sync.dma_start(
                        ot[:, 0:gn, :].rearrange("p blk f -> p (blk f)"),
                        O2[q].ap()[(lb0 // pk) * 128:
                                   (lb0 // pk + 1) * 128, :])
                    of = po.tile((128, 4, c.OUT), f32, tag="of", name="of")
                    nc.vector.tensor_copy(of[:, 0:gn, :], ot[:, 0:gn, :])
                    b0 = c.QSTART[q] + lb0
                    full = min(gn, max(0, (c.NPC // 128) - b0))
                    if full > 0:
                        nc.sync.dma_start(
                            out.ap()[b0 * 128:(b0 + full) * 128, :]
                            .rearrange("(blk p) f -> p blk f", blk=full),
                            of[:, 0:full, :])
                    for i in range(full, gn):
                        b = b0 + i
                        rows = min(128, c.NPC - b * 128)
                        if rows > 0:
                            nc.sync.dma_start(
                                out.ap()[b * 128:b * 128 + rows, :],
                                of[0:rows, i, :])

        # Layer-1 scatter -> P1 partials (pair-packed, batched per supergroup)
        with tc.tile_pool(name="pe1", bufs=4) as pe1:
            st1 = [None]

            def emit1(p, ps):
                i = p % F
                if i == 0:
                    st1[0] = pe1.tile((128, F // 2, 256), f16, name='st1')
                nc.scalar.activation(
                    st1[0][:, i // 2, (i % 2) * 128:(i % 2 + 1) * 128],
                    ps[:], copyf)
                if i == F - 1:
                    q, roff = pmeta(p - F + 1)
                    prow = roff // 2
                    nc.sync.dma_start(
                        P1[q].ap()[prow:prow + (F // 2) * 128, :]
                        .rearrange("(hi p) f -> p hi f", hi=F // 2), st1[0][:])
            def rs1(q):
                nc.gpsimd.collective_compute(
                    "ReduceScatter", mybir.AluOpType.add,
                    replica_groups=groups,
                    ins=[P1[q].ap()], outs=[H1[q].ap()])
            scatter_layer(t1, c.HID, emit1, qhook=rs1)

        with tc.tile_pool(name="ph1", bufs=4) as ph1, \
             tc.tile_pool(name="pds", bufs=4) as pds, \
             tc.tile_pool(name="pst", bufs=4, space="PSUM") as pst, \
             tc.tile_pool(name="psd", bufs=4, space="PSUM") as psd:
            for q in range(NQ):
                phase_b2_quarter(q, ph1, pds, pst, psd)

        # Layer-2 scatter -> P2 partials (quad/pair-packed rows)
        with tc.tile_pool(name="pe2", bufs=4) as pe2, \
             tc.tile_pool(name="po", bufs=3) as po:
            st2 = [None]

            def emit2(p, ps):
                q, roff = pmeta(p)
                pk = pk2[q]
                i = p % pk
                if i == 0:
                    st2[0] = pe2.tile((128, 256), f16, name='st2')
                nc.scalar.activation(
                    st2[0][:, i * c.OUT:(i + 1) * c.OUT], ps[:], copyf)
                if i == pk - 1:
                    grow = (roff // pk) - (pk - 1) * 128 // pk
                    nc.sync.dma_start(
                        P2[q].ap()[grow:grow + 128, :],
                        st2[0][:, 0:pk * c.OUT])
            def rs2(q):
                nc.gpsimd.collective_compute(
                    "ReduceScatter", mybir.AluOpType.add,
                    replica_groups=groups,
                    ins=[P2[q].ap()], outs=[O2[q].ap()])
                final_quarter(q, po)
            scatter_layer(t2, c.OUT, emit2, qhook=rs2)

    nc.compile()
    return nc


def make_inputs(cfg, features, edge_index, W1, W2):
    c = cfg
    colidx, rowloc, chunks, qoff, nch, cw, cnt128 = prep_edges(
        cfg, edge_index)
    iota2d = np.broadcast_to(np.arange(128, dtype=np.float16),
                             (128, 128)).copy()
    iotar = np.broadcast_to(np.arange(128, dtype=np.float16)[:, None],
                            (128, 8)).reshape(1, 1024)
    iotar = np.broadcast_to(iotar, (128, 1024)).copy()
    ident = np.eye(128, dtype=np.float16)
    w1 = np.ascontiguousarray(np.asarray(W1, np.float16))
    w2pad = np.zeros((c.HID, 128), np.float16)
    w2pad[:, :c.OUT] = np.asarray(W2, np.float16)
    in_maps = []
    for cc in range(c.NCORE):
        xc = np.asarray(features[cc * c.NPC:(cc + 1) * c.NPC], np.float32)
        xt = np.zeros((c.IN, c.NPP), np.float16)
        xt[:, :c.NPC] = xc.T.astype(np.float16)
        in_maps.append({
            "xT": np.ascontiguousarray(xt),
            "w1": w1, "w2p": w2pad, "iota": iota2d, "iotar": iotar,
            "ident": ident,
            "colidx": np.ascontiguousarray(colidx[cc]),
            "rowloc": np.ascontiguousarray(rowloc[cc]),
        })
    return in_maps, chunks, qoff, nch, cw, cnt128


_LAST_NC = None


def kernel(features, edge_index, W1, W2):
    global _LAST_NC
    cfg = CFG
    in_maps, chunks, qoff, nch, cw, cnt128 = make_inputs(
        cfg, features, edge_index, W1, W2)
    nc = build(cfg, chunks, qoff, nch, cw, cnt128)
    _LAST_NC = nc
    res = bass_utils.run_bass_kernel_spmd(
        nc, in_maps, core_ids=list(range(cfg.NCORE)))
    return np.concatenate(
        [res.results[cc]["out"] for cc in range(cfg.NCORE)], axis=0)

